# revision 1
# baseline (speedup 1.0000x reference)
# v4: AllGather-based K/V sharing. Each core computes Q/K/V for its own 512
# tokens; K^T and V (with softmax ones-column) are packed into one buffer and
# AllGather'd within each 4-core batch group. Attention then runs against the
# gathered full-sequence K/V in global token order (attention is invariant to
# key order, so rank-major order is fine).
import os
import numpy as np

B, S, D = 2, 2048, 1024
H, DK, DVH, DFF = 16, 64, 64, 4096
TOK = S // 4
NP = H // 2
KTILES = S // 128
KC = D // 128
MH = DFF // 128
EPS = 1e-5
BLK = NP * 512 + 4 * H * 65   # 4096 + 4160 = 8256 cols per partition row

_CACHE = {}


def _build():
    import concourse.mybir as mybir
    import concourse.tile as tile
    from concourse import bacc

    f32, f32r = mybir.dt.float32, mybir.dt.float32r
    Exp = mybir.ActivationFunctionType.Exp
    Sqrt = mybir.ActivationFunctionType.Sqrt
    Ident = mybir.ActivationFunctionType.Identity
    AX = mybir.AxisListType.X
    Alu = mybir.AluOpType

    nc = bacc.Bacc("TRN2", target_bir_lowering=False, debug=False, num_devices=8)

    xb_d = nc.dram_tensor("xb", [TOK, D], f32, kind="ExternalInput")
    wq_d = nc.dram_tensor("wq2", [D, H * DK], f32r, kind="ExternalInput")
    wk_d = nc.dram_tensor("wk2", [D, H * DK], f32r, kind="ExternalInput")
    wv_d = nc.dram_tensor("wv2", [D, H * DVH], f32r, kind="ExternalInput")
    wo_d = nc.dram_tensor("wo", [D, D], f32r, kind="ExternalInput")
    w1_d = nc.dram_tensor("w1", [D, DFF], f32r, kind="ExternalInput")
    w2_d = nc.dram_tensor("w2", [DFF, D], f32r, kind="ExternalInput")
    b1c_d = nc.dram_tensor("b1c", [128, MH], f32, kind="ExternalInput")
    b2r_d = nc.dram_tensor("b2r", [1, D], f32r, kind="ExternalInput")
    g1bc_d = nc.dram_tensor("g1bc", [128, D], f32, kind="ExternalInput")
    h1bc_d = nc.dram_tensor("h1bc", [128, D], f32, kind="ExternalInput")
    g2bc_d = nc.dram_tensor("g2bc", [128, D], f32, kind="ExternalInput")
    h2bc_d = nc.dram_tensor("h2bc", [128, D], f32, kind="ExternalInput")
    ident_d = nc.dram_tensor("ident", [128, 128], f32, kind="ExternalInput")
    ones64_d = nc.dram_tensor("ones64", [1, 64], f32r, kind="ExternalInput")
    ones128_d = nc.dram_tensor("ones128", [1, 128], f32r, kind="ExternalInput")
    onesv_d = nc.dram_tensor("onesv", [128, 64], f32r, kind="ExternalInput")
    y_d = nc.dram_tensor("y_part", [TOK, D], f32, kind="ExternalOutput")

    def ln_apply(pool, t, gbc, hbc, out_ap):
        sums = pool.tile([128, 1], f32, tag="ln_sums", name="ln_sums")
        nc.vector.reduce_sum(sums[:], t[:], axis=AX)
        sq = pool.tile([128, D], f32, tag="ln_sq", name="ln_sq")
        ssq = pool.tile([128, 1], f32, tag="ln_ssq", name="ln_ssq")
        nc.scalar.activation(
            sq[:], t[:], mybir.ActivationFunctionType.Square, accum_out=ssq[:]
        )
        s2 = pool.tile([128, 1], f32, tag="ln_s2", name="ln_s2")
        nc.vector.tensor_mul(s2[:], sums[:], sums[:])
        var0 = pool.tile([128, 1], f32, tag="ln_var0", name="ln_var0")
        nc.vector.tensor_scalar(
            out=var0[:], in0=ssq[:], scalar1=1.0 / D, scalar2=EPS,
            op0=Alu.mult, op1=Alu.add,
        )
        s2b = pool.tile([128, 1], f32, tag="ln_s2b", name="ln_s2b")
        nc.vector.tensor_scalar_mul(s2b[:], s2[:], 1.0 / (D * D))
        var = pool.tile([128, 1], f32, tag="ln_var", name="ln_var")
        nc.vector.tensor_sub(var[:], var0[:], s2b[:])
        sd = pool.tile([128, 1], f32, tag="ln_sd", name="ln_sd")
        nc.scalar.activation(sd[:], var[:], Sqrt)
        rv = pool.tile([128, 1], f32, tag="ln_rv", name="ln_rv")
        nc.vector.reciprocal(rv[:], sd[:])
        nmr = pool.tile([128, 1], f32, tag="ln_nmr", name="ln_nmr")
        nc.vector.tensor_mul(nmr[:], sums[:], rv[:])
        nmr2 = pool.tile([128, 1], f32, tag="ln_nmr2", name="ln_nmr2")
        nc.vector.tensor_scalar_mul(nmr2[:], nmr[:], -1.0 / D)
        xa = pool.tile([128, D], f32, tag="ln_xa", name="ln_xa")
        nc.scalar.activation(xa[:], t[:], Ident, bias=nmr2[:], scale=rv[:])
        xg = pool.tile([128, D], f32, tag="ln_xg", name="ln_xg")
        nc.vector.tensor_mul(xg[:], xa[:], gbc[:])
        nc.vector.tensor_add(out_ap, xg[:], hbc[:])

    with tile.TileContext(nc) as tc:
        with (
            tc.tile_pool(name="const", bufs=1) as cpool,
            tc.tile_pool(name="dram", bufs=1, space="DRAM") as dram,
        ):
            ident = cpool.tile([128, 128], f32)
            nc.sync.dma_start(ident[:], ident_d.ap())
            ones64 = cpool.tile([1, 64], f32r)
            nc.sync.dma_start(ones64[:], ones64_d.ap())
            ones128 = cpool.tile([1, 128], f32r)
            nc.sync.dma_start(ones128[:], ones128_d.ap())
            o_norm = cpool.tile([128, NP, TOK], f32r)

            k_in = dram.tile([128, NP * 512], f32r)
            k_out = dram.tile([8, 128, NP * 512], f32r, addr_space="Shared")
            v_in = dram.tile([128, 4 * H * 65], f32r)
            v_out = dram.tile([8, 128, 4 * H * 65], f32r, addr_space="Shared")

            # ones column of V goes straight into the gather input
            nc.sync.dma_start(
                v_in[:].rearrange("p (s o) -> p s o", o=65)[:, :, 64:65].squeeze(2),
                onesv_d.ap(),
            )

            with tc.tile_pool(name="mid", bufs=1) as midp:
                qT = midp.tile([128, NP, TOK], f32r)

                # ---- Phase A: transpose own x -> xT_own
                with (
                    tc.tile_pool(name="xtp", bufs=1) as xtp,
                    tc.tile_pool(name="pha", bufs=2) as pha,
                    tc.tile_pool(name="ps_tr", bufs=2, space="PSUM") as ps_tr,
                ):
                    xT = xtp.tile([128, KC, TOK], f32r)
                    for tc2 in range(2):
                        xch = pha.tile([128, 2, D], f32, tag="xch", name="xch")
                        nc.sync.dma_start(
                            xch[:],
                            xb_d.ap()[tc2 * 256:(tc2 + 1) * 256, :].rearrange(
                                "(a p) d -> p a d", p=128
                            ),
                        )
                        for dc in range(KC):
                            ps = ps_tr.tile([128, 2, 128], f32, tag="trp", name="trp")
                            for a in range(2):
                                nc.tensor.transpose(
                                    ps[:, a, :],
                                    xch[:, a, dc * 128:(dc + 1) * 128],
                                    ident[:],
                                )
                            nc.vector.tensor_copy(
                                xT[:, dc, tc2 * 256:(tc2 + 1) * 256],
                                ps[:].rearrange("p a t -> p (a t)"),
                            )

                    # ---- Phase B: Q, K, V projections on own tokens
                    with (
                        tc.tile_pool(name="wqk", bufs=3) as wqk,
                        tc.tile_pool(name="stg", bufs=4) as stg,
                        tc.tile_pool(name="ps_q", bufs=2, space="PSUM") as ps_q,
                    ):
                        # K first (feeds the collective)
                        for p in range(NP):
                            wt = wqk.tile([128, KC, 128], f32r, tag="wt", name="wt")
                            nc.sync.dma_start(
                                wt[:],
                                wk_d.ap()[:, p * 128:(p + 1) * 128].rearrange(
                                    "(kc pp) m -> pp kc m", pp=128
                                ),
                            )
                            ps = ps_q.tile([128, 512], f32, tag="psq", name="psq")
                            for kc in range(KC):
                                nc.tensor.matmul(
                                    ps[:], wt[:, kc, :], xT[:, kc, :],
                                    start=(kc == 0), stop=(kc == KC - 1),
                                )
                            st = stg.tile([128, 512], f32r, tag="kst", name="kst")
                            nc.vector.tensor_copy(st[:], ps[:])
                            nc.scalar.dma_start(
                                k_in[:, p * 512:(p + 1) * 512], st[:]
                            )
                        nc.gpsimd.collective_compute(
                            "AllGather",
                            Alu.bypass,
                            ins=[k_in.opt()],
                            outs=[k_out.opt()],
                            replica_groups=[[0, 1, 2, 3, 4, 5, 6, 7]],
                        )
                        # V (own keys)
                        wv_all = wqk.tile([128, KC, H * DVH], f32r, name="wv_all")
                        nc.sync.dma_start(
                            wv_all[:],
                            wv_d.ap().rearrange("(kc p) n -> p kc n", p=128),
                        )
                        for mtk in range(4):
                            for ncc in range(2):
                                ps = ps_q.tile([128, 512], f32, tag="psq", name="psq")
                                for kc in range(KC):
                                    nc.tensor.matmul(
                                        ps[:],
                                        xT[:, kc, mtk * 128:(mtk + 1) * 128],
                                        wv_all[:, kc, ncc * 512:(ncc + 1) * 512],
                                        start=(kc == 0), stop=(kc == KC - 1),
                                    )
                                st = stg.tile([128, 512], f32r, tag="vst", name="vst")
                                nc.vector.tensor_copy(st[:], ps[:])
                                nc.scalar.dma_start(
                                    v_in[:, mtk * 1040 + ncc * 520:]
                                    .rearrange("p (h v) -> p h v", v=65)[:, 0:8, 0:64],
                                    st[:].rearrange("p (h v) -> p h v", h=8),
                                )
                        nc.gpsimd.collective_compute(
                            "AllGather",
                            Alu.bypass,
                            ins=[v_in.opt()],
                            outs=[v_out.opt()],
                            replica_groups=[[0, 1, 2, 3, 4, 5, 6, 7]],
                        )
                        # Q projection (overlaps the collective)
                        for p in range(NP):
                            wt = wqk.tile([128, KC, 128], f32r, tag="wt", name="wt")
                            nc.sync.dma_start(
                                wt[:],
                                wq_d.ap()[:, p * 128:(p + 1) * 128].rearrange(
                                    "(kc pp) m -> pp kc m", pp=128
                                ),
                            )
                            ps = ps_q.tile([128, 512], f32, tag="psq", name="psq")
                            for kc in range(KC):
                                nc.tensor.matmul(
                                    ps[:], wt[:, kc, :], xT[:, kc, :],
                                    start=(kc == 0), stop=(kc == KC - 1),
                                )
                            nc.vector.tensor_copy(qT[:, p, :], ps[:])

                # ---- Phase C: attention against gathered K/V
                with (
                    tc.tile_pool(name="vsb", bufs=1) as vsbp,
                    tc.tile_pool(name="ktp", bufs=3) as ktpool,
                    tc.tile_pool(name="at", bufs=6) as atpool,
                    tc.tile_pool(name="rec", bufs=3) as recpool,
                    tc.tile_pool(name="ps_s", bufs=2, space="PSUM") as ps_s,
                    tc.tile_pool(name="ps_o", bufs=3, space="PSUM") as ps_o,
                    tc.tile_pool(name="ps_r", bufs=1, space="PSUM") as ps_r,
                ):
                    v_sb = vsbp.tile([128, KTILES, H, 65], f32r)
                    pid_sync = nc.sync.partition_id()
                    pid_scal = nc.scalar.partition_id()
                    for r in range(4):
                        for bb in range(2):
                            nc.sync.dma_start(
                                v_sb[:, 4 * r:4 * (r + 1), :, :],
                                v_out[4 * bb + r, :, :].rearrange(
                                    "p (t h v) -> p t h v", t=4, h=H
                                ),
                                cond=(pid_sync < 4) if bb == 0 else (4 <= pid_sync),
                            )
                    for p in range(NP):
                        ktp = ktpool.tile([128, 4, 512], f32r, tag="ktp", name="ktp")
                        for bb in range(2):
                            nc.scalar.dma_start(
                                ktp[:],
                                k_out[4 * bb:4 * (bb + 1), :,
                                      p * 512:(p + 1) * 512].transpose([1, 0, 2]),
                                cond=(pid_scal < 4) if bb == 0 else (4 <= pid_scal),
                            )
                        po = [
                            ps_o.tile([65, TOK], f32, tag="po", name=f"po{p}_{hh}")
                            for hh in range(2)
                        ]
                        for g in range(8):
                            for hh in range(2):
                                sT = ps_s.tile([128, 2, 512], f32, tag="sT", name="sT")
                                for j in range(2):
                                    kt = 2 * g + j
                                    nc.tensor.matmul(
                                        sT[:, j, :],
                                        ktp[hh * 64:(hh + 1) * 64, :, :]
                                        .rearrange("p r t -> p (r t)")[
                                            :, kt * 128:(kt + 1) * 128],
                                        qT[hh * 64:(hh + 1) * 64, p, :],
                                        tile_position=(hh * 64, 0),
                                    )
                                at = atpool.tile([128, 2, 512], f32r, tag="at", name="at")
                                nc.scalar.activation(at[:], sT[:], Exp, scale=0.125)
                                for j in range(2):
                                    kt = 2 * g + j
                                    nc.tensor.matmul(
                                        po[hh][:],
                                        v_sb[:, kt, 2 * p + hh, :],
                                        at[:, j, :],
                                        start=(kt == 0), stop=(kt == KTILES - 1),
                                    )
                        for hh in range(2):
                            rec = recpool.tile([1, TOK], f32r, tag="rec", name="rec")
                            with nc.allow_low_precision(reason="f32r"):
                                nc.vector.reciprocal(rec[:], po[hh][64:65, :])
                            rp = ps_r.tile([64, TOK], f32, tag="rp", name="rp")
                            nc.tensor.matmul(rp[:], ones64[:], rec[:])
                            rsb = recpool.tile([64, TOK], f32, tag="rsb", name="rsb")
                            nc.vector.tensor_copy(rsb[:], rp[:])
                            nc.vector.tensor_mul(
                                o_norm[hh * 64:(hh + 1) * 64, p, :],
                                po[hh][0:64, :],
                                rsb[:],
                            )

            # ---- Phase D: Wo + residual + LN1, then x1 -> x1T
            with tc.tile_pool(name="latex", bufs=1) as latex:
                with (
                    tc.tile_pool(name="wop", bufs=1) as wop,
                    tc.tile_pool(name="dstg", bufs=2) as dstg,
                    tc.tile_pool(name="lnd", bufs=2) as lnd,
                    tc.tile_pool(name="ps_wo", bufs=2, space="PSUM") as ps_wo,
                    tc.tile_pool(name="ps_t2", bufs=2, space="PSUM") as ps_t2,
                ):
                    wo_all = wop.tile([128, KC, D], f32r)
                    for kc in range(KC):
                        nc.sync.dma_start(
                            wo_all[:, kc, :],
                            wo_d.ap()[kc * 128:(kc + 1) * 128, :],
                        )
                    xosb = wop.tile([128, 4, D], f32)
                    nc.scalar.dma_start(
                        xosb[:],
                        xb_d.ap().rearrange("(a p) d -> p a d", p=128),
                    )
                    b2r = latex.tile([1, D], f32r)
                    nc.gpsimd.dma_start(b2r[:], b2r_d.ap())
                    g1bc = latex.tile([128, D], f32)
                    nc.gpsimd.dma_start(g1bc[:], g1bc_d.ap())
                    h1bc = latex.tile([128, D], f32)
                    nc.gpsimd.dma_start(h1bc[:], h1bc_d.ap())
                    g2bc = latex.tile([128, D], f32)
                    nc.gpsimd.dma_start(g2bc[:], g2bc_d.ap())
                    h2bc = latex.tile([128, D], f32)
                    nc.gpsimd.dma_start(h2bc[:], h2bc_d.ap())
                    b1c = latex.tile([128, MH], f32)
                    nc.gpsimd.dma_start(b1c[:], b1c_d.ap())
                    x1 = latex.tile([128, 4, D], f32)
                    x1T = latex.tile([128, KC, TOK], f32r)
                    for mt in range(4):
                        pso = ps_wo.tile([128, 1024], f32, tag="pso", name="pso")
                        for kc in range(KC):
                            for ncc in range(2):
                                nc.tensor.matmul(
                                    pso[:, ncc * 512:(ncc + 1) * 512],
                                    o_norm[:, kc, mt * 128:(mt + 1) * 128],
                                    wo_all[:, kc, ncc * 512:(ncc + 1) * 512],
                                    start=(kc == 0), stop=(kc == KC - 1),
                                )
                        t = dstg.tile([128, D], f32, tag="t1", name="t1")
                        for ncc in range(2):
                            nc.vector.tensor_add(
                                t[:, ncc * 512:(ncc + 1) * 512],
                                pso[:, ncc * 512:(ncc + 1) * 512],
                                xosb[:, mt, ncc * 512:(ncc + 1) * 512],
                            )
                        ln_apply(lnd, t, g1bc, h1bc, x1[:, mt, :])
                    for dc in range(KC):
                        ps = ps_t2.tile([128, 512], f32, tag="trp2", name="trp2")
                        for mt in range(4):
                            nc.tensor.transpose(
                                ps[:, mt * 128:(mt + 1) * 128],
                                x1[:, mt, dc * 128:(dc + 1) * 128],
                                ident[:],
                            )
                            nc.vector.tensor_copy(
                                x1T[:, dc, mt * 128:(mt + 1) * 128],
                                ps[:, mt * 128:(mt + 1) * 128],
                            )

                # ---- Phase E: FFN
                with (
                    tc.tile_pool(name="w1p", bufs=4) as w1p,
                    tc.tile_pool(name="ht", bufs=1) as htp,
                    tc.tile_pool(name="lne", bufs=2) as lne,
                ):
                    hT = htp.tile([128, MH, TOK], f32r)
                    with tc.tile_pool(name="ps_f1", bufs=2, space="PSUM") as ps_f1:
                        for mh in range(MH):
                            w1t = w1p.tile([128, KC, 128], f32r, tag="w1t", name="w1t")
                            (nc.sync if mh % 2 == 0 else nc.scalar).dma_start(
                                w1t[:],
                                w1_d.ap()[:, mh * 128:(mh + 1) * 128].rearrange(
                                    "(kc p) m -> p kc m", p=128
                                ),
                            )
                            ps = ps_f1.tile([128, 512], f32, tag="psf1", name="psf1")
                            for dc in range(KC):
                                nc.tensor.matmul(
                                    ps[:], w1t[:, dc, :], x1T[:, dc, :],
                                    start=(dc == 0), stop=(dc == KC - 1),
                                )
                            nc.vector.tensor_scalar(
                                out=hT[:, mh, :], in0=ps[:],
                                scalar1=b1c[:, mh:mh + 1], scalar2=0.0,
                                op0=Alu.add, op1=Alu.max,
                            )
                    with (
                        tc.tile_pool(name="w2p", bufs=4) as w2p,
                        tc.tile_pool(name="ps_f2", bufs=1, space="PSUM") as ps_f2,
                        tc.tile_pool(name="outp", bufs=2) as outp,
                    ):
                        psy = [
                            [
                                ps_f2.tile([128, 512], f32, tag=f"py{mt}{ncc}",
                                           name=f"py{mt}{ncc}")
                                for ncc in range(2)
                            ]
                            for mt in range(4)
                        ]
                        for mh in range(MH):
                            w2t = w2p.tile([128, D], f32r, tag="w2t", name="w2t")
                            (nc.sync if mh % 2 == 0 else nc.scalar).dma_start(
                                w2t[:], w2_d.ap()[mh * 128:(mh + 1) * 128, :]
                            )
                            for mt in range(4):
                                for ncc in range(2):
                                    nc.tensor.matmul(
                                        psy[mt][ncc][:],
                                        hT[:, mh, mt * 128:(mt + 1) * 128],
                                        w2t[:, ncc * 512:(ncc + 1) * 512],
                                        start=(mh == 0), stop=False,
                                    )
                        for mt in range(4):
                            for ncc in range(2):
                                nc.tensor.matmul(
                                    psy[mt][ncc][:],
                                    ones128[:],
                                    b2r[:, ncc * 512:(ncc + 1) * 512],
                                    start=False, stop=True,
                                )
                        for mt in range(4):
                            t2 = outp.tile([128, D], f32, tag="t2", name="t2")
                            for ncc in range(2):
                                nc.vector.tensor_add(
                                    t2[:, ncc * 512:(ncc + 1) * 512],
                                    psy[mt][ncc][:],
                                    x1[:, mt, ncc * 512:(ncc + 1) * 512],
                                )
                            ot = outp.tile([128, D], f32, tag="ot", name="ot")
                            ln_apply(lne, t2, g2bc, h2bc, ot[:])
                            nc.sync.dma_start(
                                y_d.ap()[mt * 128:(mt + 1) * 128, :], ot[:]
                            )
    nc.compile()
    return nc


def _in_maps(x, Wq, Wk, Wv, Wo, ln1_g, ln1_b, W1, b1, W2, b2, ln2_g, ln2_b):
    x = np.ascontiguousarray(np.asarray(x, np.float32))
    wq2 = np.ascontiguousarray(np.asarray(Wq, np.float32).transpose(1, 0, 2).reshape(D, H * DK))
    wk2 = np.ascontiguousarray(np.asarray(Wk, np.float32).transpose(1, 0, 2).reshape(D, H * DK))
    wv2 = np.ascontiguousarray(np.asarray(Wv, np.float32).transpose(1, 0, 2).reshape(D, H * DVH))
    bcast = lambda v: np.ascontiguousarray(
        np.broadcast_to(np.asarray(v, np.float32), (128, D))
    )
    common = {
        "wq2": wq2, "wk2": wk2, "wv2": wv2,
        "wo": np.ascontiguousarray(np.asarray(Wo, np.float32)),
        "w1": np.ascontiguousarray(np.asarray(W1, np.float32)),
        "w2": np.ascontiguousarray(np.asarray(W2, np.float32)),
        "b1c": np.ascontiguousarray(np.asarray(b1, np.float32).reshape(MH, 128).T),
        "b2r": np.ascontiguousarray(np.asarray(b2, np.float32).reshape(1, D)),
        "g1bc": bcast(ln1_g), "h1bc": bcast(ln1_b),
        "g2bc": bcast(ln2_g), "h2bc": bcast(ln2_b),
        "ident": np.eye(128, dtype=np.float32),
        "ones64": np.ones((1, 64), np.float32),
        "ones128": np.ones((1, 128), np.float32),
        "onesv": np.ones((128, 64), np.float32),
    }
    in_maps = []
    for c in range(8):
        b, q0 = c // 4, TOK * (c % 4)
        m = dict(common)
        m["xb"] = np.ascontiguousarray(x[b, q0:q0 + TOK, :])
        in_maps.append(m)
    return in_maps


def kernel(x, Wq, Wk, Wv, Wo, ln1_g, ln1_b, W1, b1, W2, b2, ln2_g, ln2_b):
    from concourse.bass_utils import run_bass_kernel_spmd

    if "nc" not in _CACHE:
        _CACHE["nc"] = _build()
    nc = _CACHE["nc"]
    in_maps = _in_maps(x, Wq, Wk, Wv, Wo, ln1_g, ln1_b, W1, b1, W2, b2, ln2_g, ln2_b)
    res = run_bass_kernel_spmd(nc, in_maps, core_ids=list(range(8)))
    out = np.empty((B, S, D), np.float32)
    for c in range(8):
        b, q0 = c // 4, TOK * (c % 4)
        out[b, q0:q0 + TOK, :] = res.results[c]["y_part"]
    return out



# revision 6
# speedup vs baseline: 1.2672x; 1.2672x over previous
# v5: head-parallel attention (2 heads/core over all 4096 tokens) — no K/V
# collective. Each core projects Q/K/V for its own heads from the full x^T,
# runs attention with the softmax ones-column trick, multiplies by its Wo row
# slice, and the per-core partial attn_out is combined with a ReduceScatter
# (4 pipelined 1024-token chunks overlapped with attention). After the
# scatter each core owns 512 tokens (4 groups of 128) and runs residual+LN1,
# the FFN (bf16 weights/activations, f32 accumulate), and residual+LN2.
import numpy as np

B, S, D = 2, 2048, 1024
H, DK, DVH, DFF = 16, 64, 64, 4096
N = B * S            # 4096 flattened tokens (b*S + s)
TOK = 512            # tokens owned per core after reduce-scatter
KC = D // 128        # 8
MH = DFF // 128      # 32
NG = 4               # reduce-scatter chunks (1024 tokens each)
EPS = 1e-5
NPRE = 6             # W1 chunks prefetched during attention

_CACHE = {}


def _build():
    import concourse.mybir as mybir
    import concourse.tile as tile
    from concourse import bacc

    f32, f32r = mybir.dt.float32, mybir.dt.float32r
    bf16 = mybir.dt.bfloat16
    Exp = mybir.ActivationFunctionType.Exp
    Sqrt = mybir.ActivationFunctionType.Sqrt
    Ident = mybir.ActivationFunctionType.Identity
    AX = mybir.AxisListType.X
    Alu = mybir.AluOpType

    nc = bacc.Bacc("TRN2", target_bir_lowering=False, debug=False, num_devices=8)

    xT_d = nc.dram_tensor("xT", [D, N], f32r, kind="ExternalInput")
    xs_d = nc.dram_tensor("xs", [TOK, D], f32, kind="ExternalInput")
    wq_d = nc.dram_tensor("wq", [D, 128], f32r, kind="ExternalInput")
    wk_d = nc.dram_tensor("wk", [D, 128], f32r, kind="ExternalInput")
    wv_d = nc.dram_tensor("wv", [D, 128], f32r, kind="ExternalInput")
    wo_d = nc.dram_tensor("wo", [128, D], f32r, kind="ExternalInput")
    w1_d = nc.dram_tensor("w1", [D, DFF], bf16, kind="ExternalInput")
    w2_d = nc.dram_tensor("w2", [DFF, D], bf16, kind="ExternalInput")
    b1c_d = nc.dram_tensor("b1c", [128, MH], f32, kind="ExternalInput")
    b2r_d = nc.dram_tensor("b2r", [1, D], f32r, kind="ExternalInput")
    g1bc_d = nc.dram_tensor("g1bc", [128, D], f32, kind="ExternalInput")
    h1bc_d = nc.dram_tensor("h1bc", [128, D], f32, kind="ExternalInput")
    g2bc_d = nc.dram_tensor("g2bc", [128, D], f32, kind="ExternalInput")
    h2bc_d = nc.dram_tensor("h2bc", [128, D], f32, kind="ExternalInput")
    ident_d = nc.dram_tensor("ident", [128, 128], f32, kind="ExternalInput")
    ones64_d = nc.dram_tensor("ones64", [1, 64], f32r, kind="ExternalInput")
    ones128_d = nc.dram_tensor("ones128", [1, 128], f32r, kind="ExternalInput")
    onesv_d = nc.dram_tensor("onesv", [128, 64], f32r, kind="ExternalInput")
    y_d = nc.dram_tensor("y", [TOK, D], f32, kind="ExternalOutput")

    def ln_apply(pool, t, gbc, hbc, out_ap):
        sums = pool.tile([128, 1], f32, tag="ln_sums", name="ln_sums")
        nc.vector.reduce_sum(sums[:], t[:], axis=AX)
        sq = pool.tile([128, D], f32, tag="ln_sq", name="ln_sq")
        ssq = pool.tile([128, 1], f32, tag="ln_ssq", name="ln_ssq")
        nc.scalar.activation(
            sq[:], t[:], mybir.ActivationFunctionType.Square, accum_out=ssq[:]
        )
        s2 = pool.tile([128, 1], f32, tag="ln_s2", name="ln_s2")
        nc.vector.tensor_mul(s2[:], sums[:], sums[:])
        var0 = pool.tile([128, 1], f32, tag="ln_var0", name="ln_var0")
        nc.vector.tensor_scalar(
            out=var0[:], in0=ssq[:], scalar1=1.0 / D, scalar2=EPS,
            op0=Alu.mult, op1=Alu.add,
        )
        s2b = pool.tile([128, 1], f32, tag="ln_s2b", name="ln_s2b")
        nc.vector.tensor_scalar_mul(s2b[:], s2[:], 1.0 / (D * D))
        var = pool.tile([128, 1], f32, tag="ln_var", name="ln_var")
        nc.vector.tensor_sub(var[:], var0[:], s2b[:])
        sd = pool.tile([128, 1], f32, tag="ln_sd", name="ln_sd")
        nc.scalar.activation(sd[:], var[:], Sqrt)
        rv = pool.tile([128, 1], f32, tag="ln_rv", name="ln_rv")
        nc.vector.reciprocal(rv[:], sd[:])
        nmr = pool.tile([128, 1], f32, tag="ln_nmr", name="ln_nmr")
        nc.vector.tensor_mul(nmr[:], sums[:], rv[:])
        nmr2 = pool.tile([128, 1], f32, tag="ln_nmr2", name="ln_nmr2")
        nc.vector.tensor_scalar_mul(nmr2[:], nmr[:], -1.0 / D)
        xa = pool.tile([128, D], f32, tag="ln_xa", name="ln_xa")
        nc.scalar.activation(xa[:], t[:], Ident, bias=nmr2[:], scale=rv[:])
        xg = pool.tile([128, D], f32, tag="ln_xg", name="ln_xg")
        nc.vector.tensor_mul(xg[:], xa[:], gbc[:])
        nc.vector.tensor_add(out_ap, xg[:], hbc[:])

    with tile.TileContext(nc) as tc:
        with (
            tc.tile_pool(name="const", bufs=1) as cpool,
            tc.tile_pool(name="lnp", bufs=1) as lnp,
            tc.tile_pool(name="rsp", bufs=2) as rsp,
            tc.tile_pool(name="w1p", bufs=NPRE) as w1p,
            tc.tile_pool(name="dram", bufs=1, space="DRAM") as dram,
        ):
            ident = cpool.tile([128, 128], f32)
            nc.sync.dma_start(ident[:], ident_d.ap())
            ones64 = cpool.tile([1, 64], f32r)
            nc.sync.dma_start(ones64[:], ones64_d.ap())
            ones128 = cpool.tile([1, 128], f32r)
            nc.sync.dma_start(ones128[:], ones128_d.ap())
            g1bc = cpool.tile([128, D], f32)
            nc.gpsimd.dma_start(g1bc[:], g1bc_d.ap())
            h1bc = cpool.tile([128, D], f32)
            nc.gpsimd.dma_start(h1bc[:], h1bc_d.ap())
            g2bc = cpool.tile([128, D], f32)
            nc.gpsimd.dma_start(g2bc[:], g2bc_d.ap())
            h2bc = cpool.tile([128, D], f32)
            nc.gpsimd.dma_start(h2bc[:], h2bc_d.ap())
            b1c = cpool.tile([128, MH], f32)
            nc.gpsimd.dma_start(b1c[:], b1c_d.ap())
            b2r = cpool.tile([1, D], f32r)
            nc.gpsimd.dma_start(b2r[:], b2r_d.ap())
            wo_sb = cpool.tile([128, D], f32r)
            nc.gpsimd.dma_start(wo_sb[:], wo_d.ap())
            xs = cpool.tile([128, NG, D], f32)
            nc.gpsimd.dma_start(
                xs[:], xs_d.ap().rearrange("(g p) d -> p g d", p=128)
            )
            x1 = cpool.tile([128, NG, D], f32)
            x1T = cpool.tile([128, KC, TOK], bf16)

            rs_in = [
                dram.tile([1024, D], f32, name=f"rs_in{g}") for g in range(NG)
            ]
            rs_out = [
                dram.tile([128, D], f32, name=f"rs_out{g}") for g in range(NG)
            ]

            w1pre = []
            with tc.tile_pool(name="qkt", bufs=1) as qkt:
                qT = qkt.tile([128, N], f32r)
                kT = qkt.tile([128, N], f32r)
                v_sb = qkt.tile([128, N // 128, 2, 65], f32r)
                nc.scalar.dma_start(
                    v_sb[:, :, :, 64:65].squeeze(3).rearrange("p a h -> p (a h)"),
                    onesv_d.ap(),
                )

                # ---- Phase 1: Q/K/V projections over all tokens
                with (
                    tc.tile_pool(name="wqkv", bufs=1) as wqkv,
                    tc.tile_pool(name="vtm", bufs=2) as vtm,
                    tc.tile_pool(name="xk", bufs=2) as xkp,
                    tc.tile_pool(name="ps_p", bufs=2, space="PSUM") as ps_p,
                    tc.tile_pool(name="ps_tr", bufs=2, space="PSUM") as ps_tr,
                ):
                    wq_sb = wqkv.tile([128, KC, 128], f32r)
                    nc.sync.dma_start(
                        wq_sb[:], wq_d.ap().rearrange("(kc p) m -> p kc m", p=128)
                    )
                    wk_sb = wqkv.tile([128, KC, 128], f32r)
                    nc.scalar.dma_start(
                        wk_sb[:], wk_d.ap().rearrange("(kc p) m -> p kc m", p=128)
                    )
                    wv_sb = wqkv.tile([128, KC, 128], f32r)
                    nc.gpsimd.dma_start(
                        wv_sb[:], wv_d.ap().rearrange("(kc p) m -> p kc m", p=128)
                    )
                    for tg in range(8):
                        xk = []
                        for hf in range(2):
                            xkt = xkp.tile([128, 4, 512], f32r, tag="xk",
                                           name=f"xk{tg}_{hf}")
                            eng = nc.sync if hf == 0 else nc.scalar
                            eng.dma_start(
                                xkt[:],
                                xT_d.ap()[hf * 512:(hf + 1) * 512,
                                          tg * 512:(tg + 1) * 512]
                                .rearrange("(kc p) t -> p kc t", p=128),
                            )
                            xk.append(xkt)
                        ps = ps_p.tile([128, 3, 512], f32, tag="psp", name="psp")
                        for kc in range(KC):
                            hf, kk = kc // 4, kc % 4
                            nc.tensor.matmul(
                                ps[:, 0, :], wq_sb[:, kc, :], xk[hf][:, kk, :],
                                start=(kc == 0), stop=(kc == KC - 1),
                            )
                            nc.tensor.matmul(
                                ps[:, 1, :], wk_sb[:, kc, :], xk[hf][:, kk, :],
                                start=(kc == 0), stop=(kc == KC - 1),
                            )
                            nc.tensor.matmul(
                                ps[:, 2, :], wv_sb[:, kc, :], xk[hf][:, kk, :],
                                start=(kc == 0), stop=(kc == KC - 1),
                            )
                        nc.vector.tensor_copy(
                            qT[:, tg * 512:(tg + 1) * 512], ps[:, 0, :]
                        )
                        nc.vector.tensor_copy(
                            kT[:, tg * 512:(tg + 1) * 512], ps[:, 1, :]
                        )
                        vtmp = vtm.tile([128, 512], f32, tag="vtmp", name="vtmp")
                        nc.vector.tensor_copy(vtmp[:], ps[:, 2, :])
                        for tb in range(4):
                            pt = ps_tr.tile([128, 128], f32, tag="trp", name="trp")
                            nc.tensor.transpose(
                                pt[:], vtmp[:, tb * 128:(tb + 1) * 128], ident[:]
                            )
                            nc.vector.tensor_copy(
                                v_sb[:, tg * 4 + tb, :, 0:64],
                                pt[:].rearrange("p (h v) -> p h v", h=2),
                            )

                # ---- Phase 2: attention + Wo partials + pipelined RS
                with (
                    tc.tile_pool(name="at", bufs=4) as atpool,
                    tc.tile_pool(name="onq", bufs=2) as onqp,
                    tc.tile_pool(name="wosb", bufs=2) as wosp,
                    tc.tile_pool(name="rcp", bufs=2) as rcp,
                    tc.tile_pool(name="ps_s", bufs=2, space="PSUM") as ps_s,
                    tc.tile_pool(name="ps_o", bufs=2, space="PSUM") as ps_o,
                    tc.tile_pool(name="ps_r", bufs=1, space="PSUM") as ps_r,
                    tc.tile_pool(name="ps_w", bufs=1, space="PSUM") as ps_w,
                ):
                    for qc in range(8):
                        b = qc // 4
                        po = [
                            ps_o.tile([65, 512], f32, tag="po", name=f"po{qc}_{hh}")
                            for hh in range(2)
                        ]
                        for g in range(8):
                            for hh in range(2):
                                sT = ps_s.tile([128, 2, 512], f32, tag="sT",
                                               name="sT")
                                for j in range(2):
                                    kt = 2 * g + j
                                    nc.tensor.matmul(
                                        sT[:, j, :],
                                        kT[hh * 64:(hh + 1) * 64,
                                           b * S + kt * 128:b * S + (kt + 1) * 128],
                                        qT[hh * 64:(hh + 1) * 64,
                                           qc * 512:(qc + 1) * 512],
                                        tile_position=(hh * 64, 0),
                                    )
                                at = atpool.tile([128, 2, 512], f32r, tag="at",
                                                 name="at")
                                nc.scalar.activation(at[:], sT[:], Exp, scale=0.125)
                                for j in range(2):
                                    kt = 2 * g + j
                                    nc.tensor.matmul(
                                        po[hh][:],
                                        v_sb[:, b * 16 + kt, hh, :],
                                        at[:, j, :],
                                        start=(kt == 0), stop=(kt == 15),
                                    )
                        o_nq = onqp.tile([128, 512], f32r, tag="onq", name="onq")
                        for hh in range(2):
                            rec = rcp.tile([1, 512], f32r, tag="rec", name="rec")
                            with nc.allow_low_precision(reason="f32r"):
                                nc.vector.reciprocal(rec[:], po[hh][64:65, :])
                            rp = ps_r.tile([64, 512], f32, tag="rp", name="rp")
                            nc.tensor.matmul(rp[:], ones64[:], rec[:])
                            rsb = rcp.tile([64, 512], f32, tag="rsb", name="rsb")
                            nc.vector.tensor_copy(rsb[:], rp[:])
                            nc.vector.tensor_mul(
                                o_nq[hh * 64:(hh + 1) * 64, :],
                                po[hh][0:64, :],
                                rsb[:],
                            )
                        for tc4 in range(4):
                            wos = wosp.tile([128, D], f32, tag="wos", name="wos")
                            for ncc in range(2):
                                psw = ps_w.tile([128, 512], f32, tag="psw",
                                                name="psw")
                                nc.tensor.matmul(
                                    psw[:],
                                    o_nq[:, tc4 * 128:(tc4 + 1) * 128],
                                    wo_sb[:, ncc * 512:(ncc + 1) * 512],
                                )
                                nc.vector.tensor_copy(
                                    wos[:, ncc * 512:(ncc + 1) * 512], psw[:]
                                )
                            nc.sync.dma_start(
                                rs_in[qc // 2][(qc % 2) * 512 + tc4 * 128:
                                               (qc % 2) * 512 + (tc4 + 1) * 128, :],
                                wos[:],
                            )
                        if qc % 2 == 1:
                            nc.gpsimd.collective_compute(
                                "ReduceScatter",
                                Alu.add,
                                ins=[rs_in[qc // 2].opt()],
                                outs=[rs_out[qc // 2].opt()],
                                replica_groups=[[0, 1, 2, 3, 4, 5, 6, 7]],
                            )
                        if qc >= 8 - NPRE:
                            mh = qc - (8 - NPRE)
                            w1t = w1p.tile([128, KC, 128], bf16, tag="w1t",
                                           name="w1t")
                            nc.scalar.dma_start(
                                w1t[:],
                                w1_d.ap()[:, mh * 128:(mh + 1) * 128]
                                .rearrange("(kc p) m -> p kc m", p=128),
                            )
                            w1pre.append(w1t)

            # ---- Phase 3: residual + LN1 + transpose (all 4 groups)
            with tc.tile_pool(name="ps_t2", bufs=2, space="PSUM") as ps_t2:
                for g in range(NG):
                    rst = rsp.tile([128, D], f32, tag="rst", name="rst")
                    nc.gpsimd.dma_start(rst[:], rs_out[g].opt())
                    t = rsp.tile([128, D], f32, tag="t1", name="t1")
                    nc.vector.tensor_add(t[:], rst[:], xs[:, g, :])
                    ln_apply(lnp, t, g1bc, h1bc, x1[:, g, :])
                for g in range(NG):
                    for dc in range(KC):
                        pt2 = ps_t2.tile([128, 128], f32, tag="trp2", name="trp2")
                        nc.tensor.transpose(
                            pt2[:], x1[:, g, dc * 128:(dc + 1) * 128], ident[:]
                        )
                        nc.vector.tensor_copy(
                            x1T[:, dc, g * 128:(g + 1) * 128], pt2[:]
                        )

            # ---- Phase 4: FFN (bf16 weights, f32 accumulate)
            with (
                tc.tile_pool(name="htp", bufs=1) as htp,
                tc.tile_pool(name="w2p", bufs=4) as w2p,
            ):
                hT = htp.tile([128, MH, TOK], bf16)
                w2pre = []
                with tc.tile_pool(name="ps_f1", bufs=2, space="PSUM") as ps_f1:
                    for mh in range(MH):
                        if mh < len(w1pre):
                            w1t = w1pre[mh]
                        else:
                            w1t = w1p.tile([128, KC, 128], bf16, tag="w1t",
                                           name="w1t")
                            (nc.sync if mh % 2 == 0 else nc.scalar).dma_start(
                                w1t[:],
                                w1_d.ap()[:, mh * 128:(mh + 1) * 128]
                                .rearrange("(kc p) m -> p kc m", p=128),
                            )
                        ps = ps_f1.tile([128, 512], f32, tag="psf1", name="psf1")
                        for dc in range(KC):
                            nc.tensor.matmul(
                                ps[:], w1t[:, dc, :], x1T[:, dc, :],
                                start=(dc == 0), stop=(dc == KC - 1),
                            )
                        nc.vector.tensor_scalar(
                            out=hT[:, mh, :], in0=ps[:],
                            scalar1=b1c[:, mh:mh + 1], scalar2=0.0,
                            op0=Alu.add, op1=Alu.max,
                        )
                        if mh < 3:
                            w2t = w2p.tile([128, D], bf16, tag="w2t", name="w2t")
                            nc.gpsimd.dma_start(
                                w2t[:], w2_d.ap()[mh * 128:(mh + 1) * 128, :]
                            )
                            w2pre.append(w2t)
                with (
                    tc.tile_pool(name="ps_f2", bufs=1, space="PSUM") as ps_f2,
                    tc.tile_pool(name="outp", bufs=2) as outp,
                ):
                    psy = [
                        [
                            ps_f2.tile([128, 512], f32, tag=f"py{mt}{ncc}",
                                       name=f"py{mt}{ncc}")
                            for ncc in range(2)
                        ]
                        for mt in range(4)
                    ]
                    for mh in range(MH):
                        if mh < len(w2pre):
                            w2t = w2pre[mh]
                        else:
                            w2t = w2p.tile([128, D], bf16, tag="w2t", name="w2t")
                            (nc.sync if mh % 2 == 0 else nc.scalar).dma_start(
                                w2t[:], w2_d.ap()[mh * 128:(mh + 1) * 128, :]
                            )
                        for mt in range(4):
                            for ncc in range(2):
                                nc.tensor.matmul(
                                    psy[mt][ncc][:],
                                    hT[:, mh, mt * 128:(mt + 1) * 128],
                                    w2t[:, ncc * 512:(ncc + 1) * 512],
                                    start=(mh == 0), stop=False,
                                )
                    for mt in range(4):
                        for ncc in range(2):
                            nc.tensor.matmul(
                                psy[mt][ncc][:],
                                ones128[:],
                                b2r[:, ncc * 512:(ncc + 1) * 512],
                                start=False, stop=True,
                            )
                    for mt in range(4):
                        t2 = outp.tile([128, D], f32, tag="t2", name="t2")
                        for ncc in range(2):
                            nc.vector.tensor_add(
                                t2[:, ncc * 512:(ncc + 1) * 512],
                                psy[mt][ncc][:],
                                x1[:, mt, ncc * 512:(ncc + 1) * 512],
                            )
                        ot = outp.tile([128, D], f32, tag="ot", name="ot")
                        ln_apply(lnp, t2, g2bc, h2bc, ot[:])
                        nc.sync.dma_start(
                            y_d.ap()[mt * 128:(mt + 1) * 128, :], ot[:]
                        )
    nc.compile()
    return nc


def _core_rows(c):
    return (np.arange(NG)[:, None] * 1024 + c * 128 + np.arange(128)[None, :]).ravel()


def _in_maps(x, Wq, Wk, Wv, Wo, ln1_g, ln1_b, W1, b1, W2, b2, ln2_g, ln2_b):
    import ml_dtypes

    bf16 = ml_dtypes.bfloat16
    xf = np.ascontiguousarray(np.asarray(x, np.float32).reshape(N, D))
    xT = np.ascontiguousarray(xf.T)
    Wq = np.asarray(Wq, np.float32)
    Wk = np.asarray(Wk, np.float32)
    Wv = np.asarray(Wv, np.float32)
    Wo = np.asarray(Wo, np.float32)
    bcast = lambda v: np.ascontiguousarray(
        np.broadcast_to(np.asarray(v, np.float32), (128, D))
    )
    common = {
        "xT": xT,
        "w1": np.ascontiguousarray(np.asarray(W1, np.float32).astype(bf16)),
        "w2": np.ascontiguousarray(np.asarray(W2, np.float32).astype(bf16)),
        "b1c": np.ascontiguousarray(np.asarray(b1, np.float32).reshape(MH, 128).T),
        "b2r": np.ascontiguousarray(np.asarray(b2, np.float32).reshape(1, D)),
        "g1bc": bcast(ln1_g), "h1bc": bcast(ln1_b),
        "g2bc": bcast(ln2_g), "h2bc": bcast(ln2_b),
        "ident": np.eye(128, dtype=np.float32),
        "ones64": np.ones((1, 64), np.float32),
        "ones128": np.ones((1, 128), np.float32),
        "onesv": np.ones((128, 64), np.float32),
    }
    in_maps = []
    for c in range(8):
        h0 = 2 * c
        m = dict(common)
        m["wq"] = np.ascontiguousarray(
            Wq[h0:h0 + 2].transpose(1, 0, 2).reshape(D, 128)
        )
        m["wk"] = np.ascontiguousarray(
            Wk[h0:h0 + 2].transpose(1, 0, 2).reshape(D, 128)
        )
        m["wv"] = np.ascontiguousarray(
            Wv[h0:h0 + 2].transpose(1, 0, 2).reshape(D, 128)
        )
        m["wo"] = np.ascontiguousarray(Wo[h0 * 64:h0 * 64 + 128, :])
        m["xs"] = np.ascontiguousarray(xf[_core_rows(c)])
        in_maps.append(m)
    return in_maps


def kernel(x, Wq, Wk, Wv, Wo, ln1_g, ln1_b, W1, b1, W2, b2, ln2_g, ln2_b):
    from concourse.bass_utils import run_bass_kernel_spmd

    if "nc" not in _CACHE:
        _CACHE["nc"] = _build()
    nc = _CACHE["nc"]
    in_maps = _in_maps(x, Wq, Wk, Wv, Wo, ln1_g, ln1_b, W1, b1, W2, b2, ln2_g, ln2_b)
    res = run_bass_kernel_spmd(nc, in_maps, core_ids=list(range(8)))
    out = np.empty((N, D), np.float32)
    for c in range(8):
        out[_core_rows(c)] = res.results[c]["y"]
    return out.reshape(B, S, D)


# revision 8
# speedup vs baseline: 1.3260x; 1.0464x over previous
# v6: head-parallel attention (2 heads/core over all 4096 tokens) — no K/V
# collective. Per core: project Q/K/V for its own heads from the full x^T
# (bf16 inputs, f32 accumulate), run attention software-pipelined so the PE
# never waits on the exp (PV for step k issues under the exp of step k+1),
# multiply by the core's Wo row slice, and combine partial attn_out with a
# ReduceScatter (4 pipelined 1024-token chunks overlapped with attention).
# Each core then owns 512 tokens (4 groups of 128): residual+LN1, FFN in
# bf16 split 384/128 tokens so FFN1 on the first three groups hides the last
# reduce-scatter, residual+LN2. DMAs are batched large and kept off the
# Activation queue during attention (its SEQ must keep dispatching exps).
import numpy as np

B, S, D = 2, 2048, 1024
H, DK, DVH, DFF = 16, 64, 64, 4096
N = B * S            # 4096 flattened tokens (b*S + s)
TOK = 512            # tokens owned per core after reduce-scatter
KC = D // 128        # 8
MH = DFF // 128      # 32
NG = 4               # reduce-scatter chunks (1024 tokens each)
EPS = 1e-5

_CACHE = {}


def _build():
    import concourse.mybir as mybir
    import concourse.tile as tile
    from concourse import bacc

    f32, f32r = mybir.dt.float32, mybir.dt.float32r
    bf16 = mybir.dt.bfloat16
    Exp = mybir.ActivationFunctionType.Exp
    Sqrt = mybir.ActivationFunctionType.Sqrt
    Ident = mybir.ActivationFunctionType.Identity
    AX = mybir.AxisListType.X
    Alu = mybir.AluOpType

    nc = bacc.Bacc("TRN2", target_bir_lowering=False, debug=False, num_devices=8)

    xT_d = nc.dram_tensor("xT", [D, N], bf16, kind="ExternalInput")
    xs_d = nc.dram_tensor("xs", [TOK, D], f32, kind="ExternalInput")
    wq_d = nc.dram_tensor("wq", [D, 128], bf16, kind="ExternalInput")
    wk_d = nc.dram_tensor("wk", [D, 128], bf16, kind="ExternalInput")
    wv_d = nc.dram_tensor("wv", [D, 128], bf16, kind="ExternalInput")
    wo_d = nc.dram_tensor("wo", [128, D], f32r, kind="ExternalInput")
    w1_d = nc.dram_tensor("w1", [D, DFF], bf16, kind="ExternalInput")
    w2_d = nc.dram_tensor("w2", [DFF, D], bf16, kind="ExternalInput")
    b1c_d = nc.dram_tensor("b1c", [128, MH], f32, kind="ExternalInput")
    b2r_d = nc.dram_tensor("b2r", [1, D], f32r, kind="ExternalInput")
    g1bc_d = nc.dram_tensor("g1bc", [128, D], f32, kind="ExternalInput")
    h1bc_d = nc.dram_tensor("h1bc", [128, D], f32, kind="ExternalInput")
    g2bc_d = nc.dram_tensor("g2bc", [128, D], f32, kind="ExternalInput")
    h2bc_d = nc.dram_tensor("h2bc", [128, D], f32, kind="ExternalInput")
    ident_d = nc.dram_tensor("ident", [128, 128], f32, kind="ExternalInput")
    identb_d = nc.dram_tensor("identb", [128, 128], bf16, kind="ExternalInput")
    ones64_d = nc.dram_tensor("ones64", [1, 64], f32r, kind="ExternalInput")
    ones128_d = nc.dram_tensor("ones128", [1, 128], f32r, kind="ExternalInput")
    onesv_d = nc.dram_tensor("onesv", [128, 64], f32r, kind="ExternalInput")
    y_d = nc.dram_tensor("y", [TOK, D], f32, kind="ExternalOutput")

    def ln_apply(pool, t, gbc, hbc, out_ap):
        sums = pool.tile([128, 1], f32, tag="ln_sums", name="ln_sums")
        nc.vector.reduce_sum(sums[:], t[:], axis=AX)
        # sq shares the ln_xa tag ring: it is written, never read.
        sq = pool.tile([128, D], f32, tag="ln_xa", name="ln_sq")
        ssq = pool.tile([128, 1], f32, tag="ln_ssq", name="ln_ssq")
        nc.scalar.activation(
            sq[:], t[:], mybir.ActivationFunctionType.Square, accum_out=ssq[:]
        )
        s2 = pool.tile([128, 1], f32, tag="ln_s2", name="ln_s2")
        nc.vector.tensor_mul(s2[:], sums[:], sums[:])
        var0 = pool.tile([128, 1], f32, tag="ln_var0", name="ln_var0")
        nc.vector.tensor_scalar(
            out=var0[:], in0=ssq[:], scalar1=1.0 / D, scalar2=EPS,
            op0=Alu.mult, op1=Alu.add,
        )
        s2b = pool.tile([128, 1], f32, tag="ln_s2b", name="ln_s2b")
        nc.vector.tensor_scalar_mul(s2b[:], s2[:], 1.0 / (D * D))
        var = pool.tile([128, 1], f32, tag="ln_var", name="ln_var")
        nc.vector.tensor_sub(var[:], var0[:], s2b[:])
        sd = pool.tile([128, 1], f32, tag="ln_sd", name="ln_sd")
        nc.scalar.activation(sd[:], var[:], Sqrt)
        rv = pool.tile([128, 1], f32, tag="ln_rv", name="ln_rv")
        nc.vector.reciprocal(rv[:], sd[:])
        nmr = pool.tile([128, 1], f32, tag="ln_nmr", name="ln_nmr")
        nc.vector.tensor_mul(nmr[:], sums[:], rv[:])
        nmr2 = pool.tile([128, 1], f32, tag="ln_nmr2", name="ln_nmr2")
        nc.vector.tensor_scalar_mul(nmr2[:], nmr[:], -1.0 / D)
        xa = pool.tile([128, D], f32, tag="ln_xa", name="ln_xa")
        nc.scalar.activation(xa[:], t[:], Ident, bias=nmr2[:], scale=rv[:])
        xg = pool.tile([128, D], f32, tag="ln_xg", name="ln_xg")
        nc.vector.tensor_mul(xg[:], xa[:], gbc[:])
        nc.vector.tensor_add(out_ap, xg[:], hbc[:])

    with tile.TileContext(nc) as tc:
        with (
            tc.tile_pool(name="const", bufs=1) as cpool,
            tc.tile_pool(name="lnp", bufs=2) as lnp,
            tc.tile_pool(name="rsp", bufs=2) as rsp,
            tc.tile_pool(name="w1pre", bufs=1) as w1pre_p,
            tc.tile_pool(name="dram", bufs=1, space="DRAM") as dram,
        ):
            x1 = cpool.tile([128, NG, D], bf16)
            x1T = cpool.tile([128, KC, TOK], bf16)

            rs_in = [
                dram.tile([1024, D], f32, name=f"rs_in{g}") for g in range(NG)
            ]
            rs_out = [
                dram.tile([128, D], f32, name=f"rs_out{g}") for g in range(NG)
            ]

            def x1_ln(g, xsrow, g1, h1):
                rst = rsp.tile([128, D], f32, tag="rst", name="rst")
                nc.gpsimd.dma_start(rst[:], rs_out[g].opt())
                t = rsp.tile([128, D], f32, tag="t1", name="t1")
                nc.vector.tensor_add(t[:], rst[:], xsrow)
                ln_apply(lnp, t, g1, h1, x1[:, g, :])

            w1pre = [
                w1pre_p.tile([128, KC, 512], bf16, name=f"w1pre{i}")
                for i in range(2)
            ]

            with tc.tile_pool(name="qkt", bufs=1) as qkt:
                qT = qkt.tile([128, N], f32r)
                kT = qkt.tile([128, N], f32r)
                v_sb = qkt.tile([128, N // 128, 2, 65], f32r)

                # gpsimd (SWDGE) carries all constants, ordered by first use
                wv_sb = qkt.tile([128, KC, 128], bf16)
                nc.gpsimd.dma_start(
                    wv_sb[:], wv_d.ap().rearrange("(kc p) m -> p kc m", p=128)
                )
                ident = cpool.tile([128, 128], f32)
                nc.gpsimd.dma_start(ident[:], ident_d.ap())
                nc.gpsimd.dma_start(
                    v_sb[:, :, :, 64:65].squeeze(3).rearrange("p a h -> p (a h)"),
                    onesv_d.ap(),
                )
                wo_sb = cpool.tile([128, D], f32r)
                nc.gpsimd.dma_start(wo_sb[:], wo_d.ap())
                ones64 = cpool.tile([1, 64], f32r)
                nc.gpsimd.dma_start(ones64[:], ones64_d.ap())
                ones128 = cpool.tile([1, 128], f32r)
                nc.gpsimd.dma_start(ones128[:], ones128_d.ap())
                identb = cpool.tile([128, 128], bf16)
                nc.gpsimd.dma_start(identb[:], identb_d.ap())
                b1c = cpool.tile([128, MH], f32)
                nc.gpsimd.dma_start(b1c[:], b1c_d.ap())
                b2r = cpool.tile([1, D], f32r)
                nc.gpsimd.dma_start(b2r[:], b2r_d.ap())
                g1bc = cpool.tile([128, D], f32)
                nc.gpsimd.dma_start(g1bc[:], g1bc_d.ap())
                h1bc = cpool.tile([128, D], f32)
                nc.gpsimd.dma_start(h1bc[:], h1bc_d.ap())
                xs = cpool.tile([128, NG, D], f32)
                nc.gpsimd.dma_start(
                    xs[:], xs_d.ap().rearrange("(g p) d -> p g d", p=128)
                )

                # ---- Phase 1: Q/K/V projections over all tokens
                with (
                    tc.tile_pool(name="wqk", bufs=1) as wqk,
                    tc.tile_pool(name="vtm", bufs=2) as vtm,
                    tc.tile_pool(name="xk", bufs=2) as xkp,
                    tc.tile_pool(name="ps_p", bufs=2, space="PSUM") as ps_p,
                    tc.tile_pool(name="ps_tr", bufs=2, space="PSUM") as ps_tr,
                ):
                    wq_sb = wqk.tile([128, KC, 128], bf16)
                    nc.sync.dma_start(
                        wq_sb[:], wq_d.ap().rearrange("(kc p) m -> p kc m", p=128)
                    )
                    wk_sb = wqk.tile([128, KC, 128], bf16)
                    nc.scalar.dma_start(
                        wk_sb[:], wk_d.ap().rearrange("(kc p) m -> p kc m", p=128)
                    )
                    for tg in range(8):
                        xk = xkp.tile([128, KC, 512], bf16, tag="xk",
                                      name=f"xk{tg}")
                        (nc.sync if tg % 2 == 0 else nc.scalar).dma_start(
                            xk[:],
                            xT_d.ap()[:, tg * 512:(tg + 1) * 512]
                            .rearrange("(kc p) t -> p kc t", p=128),
                        )
                        ps = ps_p.tile([128, 3, 512], f32, tag="psp", name="psp")
                        for kc in range(KC):
                            nc.tensor.matmul(
                                ps[:, 0, :], wq_sb[:, kc, :], xk[:, kc, :],
                                start=(kc == 0), stop=(kc == KC - 1),
                            )
                            nc.tensor.matmul(
                                ps[:, 1, :], wk_sb[:, kc, :], xk[:, kc, :],
                                start=(kc == 0), stop=(kc == KC - 1),
                            )
                            nc.tensor.matmul(
                                ps[:, 2, :], wv_sb[:, kc, :], xk[:, kc, :],
                                start=(kc == 0), stop=(kc == KC - 1),
                            )
                        nc.vector.tensor_copy(
                            qT[:, tg * 512:(tg + 1) * 512], ps[:, 0, :]
                        )
                        nc.vector.tensor_copy(
                            kT[:, tg * 512:(tg + 1) * 512], ps[:, 1, :]
                        )
                        vtmp = vtm.tile([128, 512], f32, tag="vtmp", name="vtmp")
                        nc.vector.tensor_copy(vtmp[:], ps[:, 2, :])
                        for tb in range(4):
                            pt = ps_tr.tile([128, 128], f32, tag="trp", name="trp")
                            nc.tensor.transpose(
                                pt[:], vtmp[:, tb * 128:(tb + 1) * 128], ident[:]
                            )
                            nc.vector.tensor_copy(
                                v_sb[:, tg * 4 + tb, :, 0:64],
                                pt[:].rearrange("p (h v) -> p h v", h=2),
                            )

                # ---- Phase 2: attention + Wo partials + pipelined RS
                with (
                    tc.tile_pool(name="at", bufs=3) as atpool,
                    tc.tile_pool(name="onq", bufs=2) as onqp,
                    tc.tile_pool(name="wosb", bufs=2) as wosp,
                    tc.tile_pool(name="rcp", bufs=1) as rcp,
                    tc.tile_pool(name="ps_s", bufs=2, space="PSUM") as ps_s,
                    tc.tile_pool(name="ps_o", bufs=2, space="PSUM") as ps_o,
                    tc.tile_pool(name="ps_r", bufs=1, space="PSUM") as ps_r,
                    tc.tile_pool(name="ps_w", bufs=1, space="PSUM") as ps_w,
                ):
                    for qc in range(8):
                        b = qc // 4
                        po = [
                            ps_o.tile([65, 512], f32, tag="po", name=f"po{qc}_{hh}")
                            for hh in range(2)
                        ]

                        def pv(prev):
                            pat, pg, phh = prev
                            for j in range(2):
                                kt = 2 * pg + j
                                nc.tensor.matmul(
                                    po[phh][:],
                                    v_sb[:, b * 16 + kt, phh, :],
                                    pat[:, j, :],
                                    start=(kt == 0), stop=(kt == 15),
                                )

                        prev = None
                        for g in range(8):
                            for hh in range(2):
                                sT = ps_s.tile([128, 2, 512], f32, tag="sT",
                                               name="sT")
                                for j in range(2):
                                    kt = 2 * g + j
                                    nc.tensor.matmul(
                                        sT[:, j, :],
                                        kT[hh * 64:(hh + 1) * 64,
                                           b * S + kt * 128:b * S + (kt + 1) * 128],
                                        qT[hh * 64:(hh + 1) * 64,
                                           qc * 512:(qc + 1) * 512],
                                        tile_position=(hh * 64, 0),
                                    )
                                at = atpool.tile([128, 2, 512], f32r, tag="at",
                                                 name="at")
                                for j in range(2):
                                    nc.scalar.activation(
                                        at[:, j, :], sT[:, j, :], Exp, scale=0.125
                                    )
                                if prev is not None:
                                    pv(prev)
                                prev = (at, g, hh)
                        pv(prev)

                        o_nq = onqp.tile([128, 512], f32r, tag="onq", name="onq")
                        for hh in range(2):
                            rec = rcp.tile([1, 512], f32r, tag="rec", name="rec")
                            with nc.allow_low_precision(reason="f32r"):
                                nc.vector.reciprocal(rec[:], po[hh][64:65, :])
                            rp = ps_r.tile([64, 512], f32, tag="rp", name="rp")
                            nc.tensor.matmul(rp[:], ones64[:], rec[:])
                            rsb = rcp.tile([64, 512], f32, tag="rsb", name="rsb")
                            nc.vector.tensor_copy(rsb[:], rp[:])
                            nc.vector.tensor_mul(
                                o_nq[hh * 64:(hh + 1) * 64, :],
                                po[hh][0:64, :],
                                rsb[:],
                            )
                        for half in range(2):
                            wos = wosp.tile([128, 2, D], f32, tag="wos",
                                            name="wos")
                            for sub in range(2):
                                tc4 = half * 2 + sub
                                for ncc in range(2):
                                    psw = ps_w.tile([128, 512], f32, tag="psw",
                                                    name="psw")
                                    nc.tensor.matmul(
                                        psw[:],
                                        o_nq[:, tc4 * 128:(tc4 + 1) * 128],
                                        wo_sb[:, ncc * 512:(ncc + 1) * 512],
                                    )
                                    nc.vector.tensor_copy(
                                        wos[:, sub, ncc * 512:(ncc + 1) * 512],
                                        psw[:],
                                    )
                            nc.sync.dma_start(
                                rs_in[qc // 2]
                                [(qc % 2) * 512 + half * 256:
                                 (qc % 2) * 512 + (half + 1) * 256, :]
                                .rearrange("(a p) d -> p a d", p=128),
                                wos[:],
                            )
                        if qc % 2 == 1:
                            nc.gpsimd.collective_compute(
                                "ReduceScatter",
                                Alu.add,
                                ins=[rs_in[qc // 2].opt()],
                                outs=[rs_out[qc // 2].opt()],
                                replica_groups=[[0, 1, 2, 3, 4, 5, 6, 7]],
                            )
                        if qc == 4:
                            x1_ln(0, xs[:, 0, :], g1bc, h1bc)
                        if qc == 6:
                            x1_ln(1, xs[:, 1, :], g1bc, h1bc)
                        if qc in (5, 6):
                            i = qc - 5
                            nc.gpsimd.dma_start(
                                w1pre[i][:],
                                w1_d.ap()[:, i * 512:(i + 1) * 512]
                                .rearrange("(kc p) m -> p kc m", p=128),
                            )

            # ---- Phase 3 + 4: LN1 for remaining groups, x1 transpose, FFN
            x1_ln(2, xs[:, 2, :], g1bc, h1bc)

            with (
                tc.tile_pool(name="w1p", bufs=4) as w1p,
                tc.tile_pool(name="htp", bufs=1) as htp,
                tc.tile_pool(name="w2p", bufs=3) as w2p,
                tc.tile_pool(name="ffc", bufs=1) as ffc,
            ):
                g2bc = ffc.tile([128, D], f32)
                nc.gpsimd.dma_start(g2bc[:], g2bc_d.ap())
                h2bc = ffc.tile([128, D], f32)
                nc.gpsimd.dma_start(h2bc[:], h2bc_d.ap())
                hT = htp.tile([128, MH, TOK], bf16)
                w2pre = []
                w1tiles = {}

                def f1b(bg, ps_f1b):
                    bt = w1tiles.pop(bg)
                    for a in range(4):
                        mh = 4 * bg + a
                        psb = ps_f1b.tile([128, 128], f32, tag="psf1b",
                                          name="psf1b")
                        for dc in range(KC):
                            nc.tensor.matmul(
                                psb[:], bt[:, dc, a * 128:(a + 1) * 128],
                                x1T[:, dc, 384:512],
                                start=(dc == 0), stop=(dc == KC - 1),
                            )
                        nc.vector.tensor_scalar(
                            out=hT[:, mh, 384:512], in0=psb[:],
                            scalar1=b1c[:, mh:mh + 1], scalar2=0.0,
                            op0=Alu.add, op1=Alu.max,
                        )

                with (
                    tc.tile_pool(name="ps_t2", bufs=2, space="PSUM") as ps_t2,
                    tc.tile_pool(name="ps_f1a", bufs=2, space="PSUM") as ps_f1a,
                    tc.tile_pool(name="ps_f1b", bufs=2, space="PSUM") as ps_f1b,
                ):
                    def x1_transpose(g):
                        for dc in range(KC):
                            pt2 = ps_t2.tile([128, 128], bf16, tag="trp2",
                                             name="trp2")
                            nc.tensor.transpose(
                                pt2[:], x1[:, g, dc * 128:(dc + 1) * 128],
                                identb[:],
                            )
                            nc.vector.tensor_copy(
                                x1T[:, dc, g * 128:(g + 1) * 128], pt2[:]
                            )

                    for g in range(3):
                        x1_transpose(g)

                    for g4 in range(8):
                        if g4 < 2:
                            w1t = w1pre[g4]
                        else:
                            w1t = w1p.tile([128, KC, 512], bf16, tag="w1t",
                                           name="w1t")
                            (nc.sync if g4 % 2 == 0 else nc.scalar).dma_start(
                                w1t[:],
                                w1_d.ap()[:, g4 * 512:(g4 + 1) * 512]
                                .rearrange("(kc p) m -> p kc m", p=128),
                            )
                        w1tiles[g4] = w1t
                        for a in range(4):
                            mh = 4 * g4 + a
                            psa = ps_f1a.tile([128, 512], f32, tag="psf1a",
                                              name="psf1a")
                            for dc in range(KC):
                                nc.tensor.matmul(
                                    psa[:, 0:384],
                                    w1t[:, dc, a * 128:(a + 1) * 128],
                                    x1T[:, dc, 0:384],
                                    start=(dc == 0), stop=(dc == KC - 1),
                                )
                            nc.vector.tensor_scalar(
                                out=hT[:, mh, 0:384], in0=psa[:, 0:384],
                                scalar1=b1c[:, mh:mh + 1], scalar2=0.0,
                                op0=Alu.add, op1=Alu.max,
                            )
                        if g4 == 3:
                            x1_ln(3, xs[:, 3, :], g1bc, h1bc)
                            x1_transpose(3)
                        if g4 >= 3:
                            f1b(g4 - 3, ps_f1b)
                        if g4 in (5, 6):
                            w2t = w2p.tile([128, 2, D], bf16, tag="w2t",
                                           name="w2t")
                            nc.gpsimd.dma_start(
                                w2t[:],
                                w2_d.ap()[(g4 - 5) * 256:(g4 - 4) * 256, :]
                                .rearrange("(a p) d -> p a d", p=128),
                            )
                            w2pre.append(w2t)
                    for bg in range(5, 8):
                        f1b(bg, ps_f1b)

                # ---- FFN2 + residual + LN2
                with (
                    tc.tile_pool(name="ps_f2", bufs=1, space="PSUM") as ps_f2,
                    tc.tile_pool(name="outp", bufs=1) as outp,
                ):
                    psy = [
                        [
                            ps_f2.tile([128, 512], f32, tag=f"py{mt}{ncc}",
                                       name=f"py{mt}{ncc}")
                            for ncc in range(2)
                        ]
                        for mt in range(4)
                    ]
                    for wc in range(16):
                        if wc < len(w2pre):
                            w2t = w2pre[wc]
                        else:
                            w2t = w2p.tile([128, 2, D], bf16, tag="w2t",
                                           name="w2t")
                            (nc.sync if wc % 2 == 0 else nc.scalar).dma_start(
                                w2t[:],
                                w2_d.ap()[wc * 256:(wc + 1) * 256, :]
                                .rearrange("(a p) d -> p a d", p=128),
                            )
                        for a in range(2):
                            mh = 2 * wc + a
                            for mt in range(4):
                                for ncc in range(2):
                                    nc.tensor.matmul(
                                        psy[mt][ncc][:],
                                        hT[:, mh, mt * 128:(mt + 1) * 128],
                                        w2t[:, a, ncc * 512:(ncc + 1) * 512],
                                        start=(mh == 0), stop=False,
                                    )
                    for mt in range(4):
                        for ncc in range(2):
                            nc.tensor.matmul(
                                psy[mt][ncc][:],
                                ones128[:],
                                b2r[:, ncc * 512:(ncc + 1) * 512],
                                start=False, stop=True,
                            )
                        t2 = outp.tile([128, D], f32, tag="t2", name="t2")
                        for ncc in range(2):
                            nc.vector.tensor_add(
                                t2[:, ncc * 512:(ncc + 1) * 512],
                                psy[mt][ncc][:],
                                x1[:, mt, ncc * 512:(ncc + 1) * 512],
                            )
                        ot = outp.tile([128, D], f32, tag="ot", name="ot")
                        ln_apply(lnp, t2, g2bc, h2bc, ot[:])
                        nc.sync.dma_start(
                            y_d.ap()[mt * 128:(mt + 1) * 128, :], ot[:]
                        )
    nc.compile()
    return nc


def _core_rows(c):
    return (np.arange(NG)[:, None] * 1024 + c * 128 + np.arange(128)[None, :]).ravel()


def _in_maps(x, Wq, Wk, Wv, Wo, ln1_g, ln1_b, W1, b1, W2, b2, ln2_g, ln2_b):
    import ml_dtypes

    bf16 = ml_dtypes.bfloat16
    xf = np.ascontiguousarray(np.asarray(x, np.float32).reshape(N, D))
    xT = np.ascontiguousarray(xf.T.astype(bf16))
    Wq = np.asarray(Wq, np.float32)
    Wk = np.asarray(Wk, np.float32)
    Wv = np.asarray(Wv, np.float32)
    Wo = np.asarray(Wo, np.float32)
    bcast = lambda v: np.ascontiguousarray(
        np.broadcast_to(np.asarray(v, np.float32), (128, D))
    )
    common = {
        "xT": xT,
        "w1": np.ascontiguousarray(np.asarray(W1, np.float32).astype(bf16)),
        "w2": np.ascontiguousarray(np.asarray(W2, np.float32).astype(bf16)),
        "b1c": np.ascontiguousarray(np.asarray(b1, np.float32).reshape(MH, 128).T),
        "b2r": np.ascontiguousarray(np.asarray(b2, np.float32).reshape(1, D)),
        "g1bc": bcast(ln1_g), "h1bc": bcast(ln1_b),
        "g2bc": bcast(ln2_g), "h2bc": bcast(ln2_b),
        "ident": np.eye(128, dtype=np.float32),
        "identb": np.eye(128, dtype=np.float32).astype(bf16),
        "ones64": np.ones((1, 64), np.float32),
        "ones128": np.ones((1, 128), np.float32),
        "onesv": np.ones((128, 64), np.float32),
    }
    in_maps = []
    for c in range(8):
        h0 = 2 * c
        m = dict(common)
        m["wq"] = np.ascontiguousarray(
            Wq[h0:h0 + 2].transpose(1, 0, 2).reshape(D, 128).astype(bf16)
        )
        m["wk"] = np.ascontiguousarray(
            Wk[h0:h0 + 2].transpose(1, 0, 2).reshape(D, 128).astype(bf16)
        )
        m["wv"] = np.ascontiguousarray(
            Wv[h0:h0 + 2].transpose(1, 0, 2).reshape(D, 128).astype(bf16)
        )
        m["wo"] = np.ascontiguousarray(Wo[h0 * 64:h0 * 64 + 128, :])
        m["xs"] = np.ascontiguousarray(xf[_core_rows(c)])
        in_maps.append(m)
    return in_maps


def kernel(x, Wq, Wk, Wv, Wo, ln1_g, ln1_b, W1, b1, W2, b2, ln2_g, ln2_b):
    from concourse.bass_utils import run_bass_kernel_spmd

    if "nc" not in _CACHE:
        _CACHE["nc"] = _build()
    nc = _CACHE["nc"]
    in_maps = _in_maps(x, Wq, Wk, Wv, Wo, ln1_g, ln1_b, W1, b1, W2, b2, ln2_g, ln2_b)
    res = run_bass_kernel_spmd(nc, in_maps, core_ids=list(range(8)))
    out = np.empty((N, D), np.float32)
    for c in range(8):
        out[_core_rows(c)] = res.results[c]["y"]
    return out.reshape(B, S, D)


# revision 12
# speedup vs baseline: 1.4105x; 1.0637x over previous
# v6: head-parallel attention (2 heads/core over all 4096 tokens) — no K/V
# collective. Per core: project Q/K/V for its own heads from the full x^T
# (bf16 inputs, f32 accumulate), run attention software-pipelined so the PE
# never waits on the exp (PV for step k issues under the exp of step k+1),
# multiply by the core's Wo row slice, and combine partial attn_out with a
# ReduceScatter (4 pipelined 1024-token chunks overlapped with attention).
# Each core then owns 512 tokens (4 groups of 128): residual+LN1, FFN in
# bf16 split 384/128 tokens so FFN1 on the first three groups hides the last
# reduce-scatter, residual+LN2. DMAs are batched large and kept off the
# Activation queue during attention (its SEQ must keep dispatching exps).
import numpy as np

B, S, D = 2, 2048, 1024
H, DK, DVH, DFF = 16, 64, 64, 4096
N = B * S            # 4096 flattened tokens (b*S + s)
TOK = 512            # tokens owned per core after reduce-scatter
KC = D // 128        # 8
MH = DFF // 128      # 32
NG = 4               # reduce-scatter chunks (1024 tokens each)
EPS = 1e-5

_CACHE = {}


def _build():
    import concourse.mybir as mybir
    import concourse.tile as tile
    from concourse import bacc

    f32, f32r = mybir.dt.float32, mybir.dt.float32r
    bf16 = mybir.dt.bfloat16
    Exp = mybir.ActivationFunctionType.Exp
    Sqrt = mybir.ActivationFunctionType.Sqrt
    Ident = mybir.ActivationFunctionType.Identity
    AX = mybir.AxisListType.X
    Alu = mybir.AluOpType

    nc = bacc.Bacc("TRN2", target_bir_lowering=False, debug=False, num_devices=8)

    xT_d = nc.dram_tensor("xT", [D, N], bf16, kind="ExternalInput")
    xs_d = nc.dram_tensor("xs", [TOK, D], f32, kind="ExternalInput")
    wq_d = nc.dram_tensor("wq", [D, 128], bf16, kind="ExternalInput")
    wk_d = nc.dram_tensor("wk", [D, 128], bf16, kind="ExternalInput")
    wv_d = nc.dram_tensor("wv", [D, 128], bf16, kind="ExternalInput")
    wo_d = nc.dram_tensor("wo", [128, D], f32r, kind="ExternalInput")
    w1_d = nc.dram_tensor("w1", [D, DFF], bf16, kind="ExternalInput")
    w2_d = nc.dram_tensor("w2", [DFF, D], bf16, kind="ExternalInput")
    b1c_d = nc.dram_tensor("b1c", [128, MH], f32, kind="ExternalInput")
    b2r_d = nc.dram_tensor("b2r", [1, D], f32r, kind="ExternalInput")
    g1bc_d = nc.dram_tensor("g1bc", [128, D], f32, kind="ExternalInput")
    h1bc_d = nc.dram_tensor("h1bc", [128, D], f32, kind="ExternalInput")
    g2bc_d = nc.dram_tensor("g2bc", [128, D], f32, kind="ExternalInput")
    h2bc_d = nc.dram_tensor("h2bc", [128, D], f32, kind="ExternalInput")
    ident_d = nc.dram_tensor("ident", [128, 128], f32, kind="ExternalInput")
    identb_d = nc.dram_tensor("identb", [128, 128], bf16, kind="ExternalInput")
    ones128_d = nc.dram_tensor("ones128", [1, 128], f32r, kind="ExternalInput")
    onesv_d = nc.dram_tensor("onesv", [128, 64], f32r, kind="ExternalInput")
    y_d = nc.dram_tensor("y", [TOK, D], f32, kind="ExternalOutput")

    def ln_apply(pool, t, gbc, hbc, out_ap):
        sums = pool.tile([128, 1], f32, tag="ln_sums", name="ln_sums")
        nc.vector.reduce_sum(sums[:], t[:], axis=AX)
        # sq shares the ln_xa tag ring: it is written, never read.
        sq = pool.tile([128, D], f32, tag="ln_xa", name="ln_sq")
        ssq = pool.tile([128, 1], f32, tag="ln_ssq", name="ln_ssq")
        nc.scalar.activation(
            sq[:], t[:], mybir.ActivationFunctionType.Square, accum_out=ssq[:]
        )
        s2 = pool.tile([128, 1], f32, tag="ln_s2", name="ln_s2")
        nc.vector.tensor_mul(s2[:], sums[:], sums[:])
        var0 = pool.tile([128, 1], f32, tag="ln_var0", name="ln_var0")
        nc.vector.tensor_scalar(
            out=var0[:], in0=ssq[:], scalar1=1.0 / D, scalar2=EPS,
            op0=Alu.mult, op1=Alu.add,
        )
        s2b = pool.tile([128, 1], f32, tag="ln_s2b", name="ln_s2b")
        nc.vector.tensor_scalar_mul(s2b[:], s2[:], 1.0 / (D * D))
        var = pool.tile([128, 1], f32, tag="ln_var", name="ln_var")
        nc.vector.tensor_sub(var[:], var0[:], s2b[:])
        sd = pool.tile([128, 1], f32, tag="ln_sd", name="ln_sd")
        nc.scalar.activation(sd[:], var[:], Sqrt)
        rv = pool.tile([128, 1], f32, tag="ln_rv", name="ln_rv")
        nc.vector.reciprocal(rv[:], sd[:])
        nmr = pool.tile([128, 1], f32, tag="ln_nmr", name="ln_nmr")
        nc.vector.tensor_mul(nmr[:], sums[:], rv[:])
        nmr2 = pool.tile([128, 1], f32, tag="ln_nmr2", name="ln_nmr2")
        nc.vector.tensor_scalar_mul(nmr2[:], nmr[:], -1.0 / D)
        xa = pool.tile([128, D], f32, tag="ln_xa", name="ln_xa")
        nc.scalar.activation(xa[:], t[:], Ident, bias=nmr2[:], scale=rv[:])
        xg = pool.tile([128, D], f32, tag="ln_xg", name="ln_xg")
        nc.vector.tensor_mul(xg[:], xa[:], gbc[:])
        nc.vector.tensor_add(out_ap, xg[:], hbc[:])

    with tile.TileContext(nc) as tc:
        with (
            tc.tile_pool(name="const", bufs=1) as cpool,
            tc.tile_pool(name="lnp", bufs=2) as lnp,
            tc.tile_pool(name="rsp", bufs=2) as rsp,
            tc.tile_pool(name="w1pre", bufs=1) as w1pre_p,
            tc.tile_pool(name="dram", bufs=1, space="DRAM") as dram,
        ):
            x1 = cpool.tile([128, NG, D], bf16)
            x1T = cpool.tile([128, KC, TOK], bf16)

            rs_in = [
                dram.tile([1024, D], f32, name=f"rs_in{g}") for g in range(NG)
            ]
            rs_out = [
                dram.tile([128, D], f32, name=f"rs_out{g}") for g in range(NG)
            ]

            def x1_ln(g, xsrow, g1, h1):
                rst = rsp.tile([128, D], f32, tag="rst", name="rst")
                nc.gpsimd.dma_start(rst[:], rs_out[g].opt())
                t = rsp.tile([128, D], f32, tag="t1", name="t1")
                nc.vector.tensor_add(t[:], rst[:], xsrow)
                ln_apply(lnp, t, g1, h1, x1[:, g, :])

            w1pre = [
                w1pre_p.tile([128, KC, 512], bf16, name=f"w1pre{i}")
                for i in range(2)
            ]

            with tc.tile_pool(name="qkt", bufs=1) as qkt:
                qT = qkt.tile([128, N], f32r)
                kT = qkt.tile([128, N], f32r)
                v_sb = qkt.tile([128, N // 128, 2, 65], f32r)

                # gpsimd (SWDGE) carries all constants, ordered by first use
                wv_sb = qkt.tile([128, KC, 128], bf16)
                nc.gpsimd.dma_start(
                    wv_sb[:], wv_d.ap().rearrange("(kc p) m -> p kc m", p=128)
                )
                ident = cpool.tile([128, 128], f32)
                nc.gpsimd.dma_start(ident[:], ident_d.ap())
                nc.gpsimd.dma_start(
                    v_sb[:, :, :, 64:65].squeeze(3).rearrange("p a h -> p (a h)"),
                    onesv_d.ap(),
                )
                wo_sb = cpool.tile([128, D], f32r)
                nc.gpsimd.dma_start(wo_sb[:], wo_d.ap())
                ones128 = cpool.tile([1, 128], f32r)
                nc.gpsimd.dma_start(ones128[:], ones128_d.ap())
                identb = cpool.tile([128, 128], bf16)
                nc.gpsimd.dma_start(identb[:], identb_d.ap())
                b1c = cpool.tile([128, MH], f32)
                nc.gpsimd.dma_start(b1c[:], b1c_d.ap())
                b2r = cpool.tile([1, D], f32r)
                nc.gpsimd.dma_start(b2r[:], b2r_d.ap())
                g1bc = cpool.tile([128, D], f32)
                nc.gpsimd.dma_start(g1bc[:], g1bc_d.ap())
                h1bc = cpool.tile([128, D], f32)
                nc.gpsimd.dma_start(h1bc[:], h1bc_d.ap())
                xs = cpool.tile([128, NG, D], f32)
                nc.gpsimd.dma_start(
                    xs[:], xs_d.ap().rearrange("(g p) d -> p g d", p=128)
                )

                # ---- Phase 1: Q/K/V projections over all tokens
                with (
                    tc.tile_pool(name="wqk", bufs=1) as wqk,
                    tc.tile_pool(name="vtm", bufs=2) as vtm,
                    tc.tile_pool(name="xk", bufs=2) as xkp,
                    tc.tile_pool(name="ps_p", bufs=2, space="PSUM") as ps_p,
                    tc.tile_pool(name="ps_tr", bufs=2, space="PSUM") as ps_tr,
                ):
                    wq_sb = wqk.tile([128, KC, 128], bf16)
                    nc.sync.dma_start(
                        wq_sb[:], wq_d.ap().rearrange("(kc p) m -> p kc m", p=128)
                    )
                    wk_sb = wqk.tile([128, KC, 128], bf16)
                    nc.scalar.dma_start(
                        wk_sb[:], wk_d.ap().rearrange("(kc p) m -> p kc m", p=128)
                    )
                    for tg in range(8):
                        xk = xkp.tile([128, KC, 512], bf16, tag="xk",
                                      name=f"xk{tg}")
                        (nc.sync if tg % 2 == 0 else nc.scalar).dma_start(
                            xk[:],
                            xT_d.ap()[:, tg * 512:(tg + 1) * 512]
                            .rearrange("(kc p) t -> p kc t", p=128),
                        )
                        ps = ps_p.tile([128, 3, 512], f32, tag="psp", name="psp")
                        for kc in range(KC):
                            nc.tensor.matmul(
                                ps[:, 0, :], wq_sb[:, kc, :], xk[:, kc, :],
                                start=(kc == 0), stop=(kc == KC - 1),
                            )
                            nc.tensor.matmul(
                                ps[:, 1, :], wk_sb[:, kc, :], xk[:, kc, :],
                                start=(kc == 0), stop=(kc == KC - 1),
                            )
                            nc.tensor.matmul(
                                ps[:, 2, :], wv_sb[:, kc, :], xk[:, kc, :],
                                start=(kc == 0), stop=(kc == KC - 1),
                            )
                        nc.vector.tensor_copy(
                            qT[:, tg * 512:(tg + 1) * 512], ps[:, 0, :]
                        )
                        nc.vector.tensor_copy(
                            kT[:, tg * 512:(tg + 1) * 512], ps[:, 1, :]
                        )
                        vtmp = vtm.tile([128, 512], f32, tag="vtmp", name="vtmp")
                        nc.vector.tensor_copy(vtmp[:], ps[:, 2, :])
                        for tb in range(4):
                            pt = ps_tr.tile([128, 128], f32, tag="trp", name="trp")
                            nc.tensor.transpose(
                                pt[:], vtmp[:, tb * 128:(tb + 1) * 128], ident[:]
                            )
                            nc.vector.tensor_copy(
                                v_sb[:, tg * 4 + tb, :, 0:64],
                                pt[:].rearrange("p (h v) -> p h v", h=2),
                            )

                # ---- Phase 2: attention + Wo partials + pipelined RS
                with (
                    tc.tile_pool(name="at", bufs=3) as atpool,
                    tc.tile_pool(name="onq", bufs=2) as onqp,
                    tc.tile_pool(name="wosb", bufs=2) as wosp,
                    tc.tile_pool(name="rcp", bufs=1) as rcp,
                    tc.tile_pool(name="ps_s", bufs=2, space="PSUM") as ps_s,
                    tc.tile_pool(name="ps_o", bufs=3, space="PSUM") as ps_o,
                    tc.tile_pool(name="ps_w", bufs=1, space="PSUM") as ps_w,
                ):
                    for qc in range(8):
                        b = qc // 4
                        po = [
                            ps_o.tile([65, 512], f32, tag="po", name=f"po{qc}_{hh}")
                            for hh in range(2)
                        ]

                        def pv(prev):
                            pat, pg, phh = prev
                            for j in range(2):
                                kt = 2 * pg + j
                                nc.tensor.matmul(
                                    po[phh][:],
                                    v_sb[:, b * 16 + kt, phh, :],
                                    pat[:, j, :],
                                    start=(kt == 0), stop=(kt == 15),
                                )

                        prev = None
                        for g in range(8):
                            for hh in range(2):
                                sT = ps_s.tile([128, 2, 512], f32, tag="sT",
                                               name="sT")
                                for j in range(2):
                                    kt = 2 * g + j
                                    nc.tensor.matmul(
                                        sT[:, j, :],
                                        kT[hh * 64:(hh + 1) * 64,
                                           b * S + kt * 128:b * S + (kt + 1) * 128],
                                        qT[hh * 64:(hh + 1) * 64,
                                           qc * 512:(qc + 1) * 512],
                                        tile_position=(hh * 64, 0),
                                    )
                                at = atpool.tile([128, 2, 512], f32r, tag="at",
                                                 name="at")
                                nc.scalar.activation(
                                    at[:], sT[:], Exp, scale=0.125
                                )
                                if prev is not None:
                                    pv(prev)
                                prev = (at, g, hh)
                        pv(prev)

                        o_nq = onqp.tile([128, 512], f32r, tag="onq", name="onq")
                        for hh in range(2):
                            rec = rcp.tile([1, 512], f32r, tag="rec", name="rec")
                            with nc.allow_low_precision(reason="f32r"):
                                nc.vector.reciprocal(rec[:], po[hh][64:65, :])
                            rsb = rcp.tile([64, 512], f32r, tag="rsb", name="rsb")
                            nc.gpsimd.partition_broadcast(rsb[:], rec[:])
                            nc.vector.tensor_mul(
                                o_nq[hh * 64:(hh + 1) * 64, :],
                                po[hh][0:64, :],
                                rsb[:],
                            )
                        for half in range(2):
                            wos = wosp.tile([128, 2, D], f32, tag="wos",
                                            name="wos")
                            for sub in range(2):
                                tc4 = half * 2 + sub
                                for ncc in range(2):
                                    psw = ps_w.tile([128, 512], f32, tag="psw",
                                                    name="psw")
                                    nc.tensor.matmul(
                                        psw[:],
                                        o_nq[:, tc4 * 128:(tc4 + 1) * 128],
                                        wo_sb[:, ncc * 512:(ncc + 1) * 512],
                                    )
                                    nc.vector.tensor_copy(
                                        wos[:, sub, ncc * 512:(ncc + 1) * 512],
                                        psw[:],
                                    )
                            nc.sync.dma_start(
                                rs_in[qc // 2]
                                [(qc % 2) * 512 + half * 256:
                                 (qc % 2) * 512 + (half + 1) * 256, :]
                                .rearrange("(a p) d -> p a d", p=128),
                                wos[:],
                            )
                        if qc % 2 == 1:
                            nc.gpsimd.collective_compute(
                                "ReduceScatter",
                                Alu.add,
                                ins=[rs_in[qc // 2].opt()],
                                outs=[rs_out[qc // 2].opt()],
                                replica_groups=[[0, 1, 2, 3, 4, 5, 6, 7]],
                            )
                        if qc in (5, 6):
                            i = qc - 5
                            nc.gpsimd.dma_start(
                                w1pre[i][:],
                                w1_d.ap()[:, i * 512:(i + 1) * 512]
                                .rearrange("(kc p) m -> p kc m", p=128),
                            )

            # ---- Phase 3 + 4: LN1 (groups 0-2), x1 transpose, FFN

            with (
                tc.tile_pool(name="w1p", bufs=4) as w1p,
                tc.tile_pool(name="htp", bufs=1) as htp,
                tc.tile_pool(name="w2p", bufs=3) as w2p,
                tc.tile_pool(name="ffc", bufs=1) as ffc,
            ):
                g2bc = ffc.tile([128, D], f32)
                nc.gpsimd.dma_start(g2bc[:], g2bc_d.ap())
                h2bc = ffc.tile([128, D], f32)
                nc.gpsimd.dma_start(h2bc[:], h2bc_d.ap())
                hT = htp.tile([128, MH, TOK], bf16)
                w2pre = []
                w1tiles = {}

                def f1b(bg, ps_f1b):
                    bt = w1tiles.pop(bg)
                    for a in range(4):
                        mh = 4 * bg + a
                        psb = ps_f1b.tile([128, 128], f32, tag="psf1b",
                                          name="psf1b")
                        for dc in range(KC):
                            nc.tensor.matmul(
                                psb[:], bt[:, dc, a * 128:(a + 1) * 128],
                                x1T[:, dc, 384:512],
                                start=(dc == 0), stop=(dc == KC - 1),
                            )
                        nc.vector.tensor_scalar(
                            out=hT[:, mh, 384:512], in0=psb[:],
                            scalar1=b1c[:, mh:mh + 1], scalar2=0.0,
                            op0=Alu.add, op1=Alu.max,
                        )

                with (
                    tc.tile_pool(name="ps_t2", bufs=2, space="PSUM") as ps_t2,
                    tc.tile_pool(name="ps_f1a", bufs=2, space="PSUM") as ps_f1a,
                    tc.tile_pool(name="ps_f1b", bufs=2, space="PSUM") as ps_f1b,
                ):
                    def x1_transpose(g):
                        for dc in range(KC):
                            pt2 = ps_t2.tile([128, 128], bf16, tag="trp2",
                                             name="trp2")
                            nc.tensor.transpose(
                                pt2[:], x1[:, g, dc * 128:(dc + 1) * 128],
                                identb[:],
                            )
                            nc.vector.tensor_copy(
                                x1T[:, dc, g * 128:(g + 1) * 128], pt2[:]
                            )

                    for g in range(3):
                        x1_ln(g, xs[:, g, :], g1bc, h1bc)
                        x1_transpose(g)

                    for g4 in range(8):
                        if g4 < 2:
                            w1t = w1pre[g4]
                        else:
                            w1t = w1p.tile([128, KC, 512], bf16, tag="w1t",
                                           name="w1t")
                            (nc.sync if g4 % 2 == 0 else nc.scalar).dma_start(
                                w1t[:],
                                w1_d.ap()[:, g4 * 512:(g4 + 1) * 512]
                                .rearrange("(kc p) m -> p kc m", p=128),
                            )
                        w1tiles[g4] = w1t
                        for a in range(4):
                            mh = 4 * g4 + a
                            psa = ps_f1a.tile([128, 512], f32, tag="psf1a",
                                              name="psf1a")
                            for dc in range(KC):
                                nc.tensor.matmul(
                                    psa[:, 0:384],
                                    w1t[:, dc, a * 128:(a + 1) * 128],
                                    x1T[:, dc, 0:384],
                                    start=(dc == 0), stop=(dc == KC - 1),
                                )
                            nc.vector.tensor_scalar(
                                out=hT[:, mh, 0:384], in0=psa[:, 0:384],
                                scalar1=b1c[:, mh:mh + 1], scalar2=0.0,
                                op0=Alu.add, op1=Alu.max,
                            )
                        if g4 == 3:
                            x1_ln(3, xs[:, 3, :], g1bc, h1bc)
                            x1_transpose(3)
                        if g4 >= 3:
                            f1b(g4 - 3, ps_f1b)
                        if g4 == 6:
                            w2t = w2p.tile([128, 4, D], bf16, tag="w2t",
                                           name="w2t")
                            nc.gpsimd.dma_start(
                                w2t[:],
                                w2_d.ap()[0:512, :]
                                .rearrange("(a p) d -> p a d", p=128),
                            )
                            w2pre.append(w2t)
                    for bg in range(5, 8):
                        f1b(bg, ps_f1b)

                # ---- FFN2 + residual + LN2
                with (
                    tc.tile_pool(name="ps_f2", bufs=1, space="PSUM") as ps_f2,
                    tc.tile_pool(name="outp", bufs=1) as outp,
                ):
                    psy = [
                        [
                            ps_f2.tile([128, 512], f32, tag=f"py{mt}{ncc}",
                                       name=f"py{mt}{ncc}")
                            for ncc in range(2)
                        ]
                        for mt in range(4)
                    ]
                    def f2_finish(mt):
                        for ncc in range(2):
                            nc.tensor.matmul(
                                psy[mt][ncc][:],
                                ones128[:],
                                b2r[:, ncc * 512:(ncc + 1) * 512],
                                start=False, stop=True,
                            )
                        t2 = outp.tile([128, D], f32, tag="t2", name="t2")
                        for ncc in range(2):
                            nc.vector.tensor_add(
                                t2[:, ncc * 512:(ncc + 1) * 512],
                                psy[mt][ncc][:],
                                x1[:, mt, ncc * 512:(ncc + 1) * 512],
                            )
                        ot = outp.tile([128, D], f32, tag="ot", name="ot")
                        ln_apply(lnp, t2, g2bc, h2bc, ot[:])
                        nc.sync.dma_start(
                            y_d.ap()[mt * 128:(mt + 1) * 128, :], ot[:]
                        )

                    for p2 in range(2):
                        for wc in range(8):
                            if p2 == 0 and wc < len(w2pre):
                                w2t = w2pre[wc]
                            else:
                                w2t = w2p.tile([128, 4, D], bf16, tag="w2t",
                                               name="w2t")
                                (nc.sync if wc % 2 == 0 else nc.scalar).dma_start(
                                    w2t[:],
                                    w2_d.ap()[wc * 512:(wc + 1) * 512, :]
                                    .rearrange("(a p) d -> p a d", p=128),
                                )
                            for a in range(4):
                                mh = 4 * wc + a
                                for mt in (2 * p2, 2 * p2 + 1):
                                    for ncc in range(2):
                                        nc.tensor.matmul(
                                            psy[mt][ncc][:],
                                            hT[:, mh, mt * 128:(mt + 1) * 128],
                                            w2t[:, a, ncc * 512:(ncc + 1) * 512],
                                            start=(mh == 0), stop=False,
                                        )
                        f2_finish(2 * p2)
                        f2_finish(2 * p2 + 1)
    nc.compile()
    return nc


def _core_rows(c):
    return (np.arange(NG)[:, None] * 1024 + c * 128 + np.arange(128)[None, :]).ravel()


def _in_maps(x, Wq, Wk, Wv, Wo, ln1_g, ln1_b, W1, b1, W2, b2, ln2_g, ln2_b):
    import ml_dtypes

    bf16 = ml_dtypes.bfloat16
    xf = np.ascontiguousarray(np.asarray(x, np.float32).reshape(N, D))
    xT = np.ascontiguousarray(xf.T.astype(bf16))
    Wq = np.asarray(Wq, np.float32)
    Wk = np.asarray(Wk, np.float32)
    Wv = np.asarray(Wv, np.float32)
    Wo = np.asarray(Wo, np.float32)
    bcast = lambda v: np.ascontiguousarray(
        np.broadcast_to(np.asarray(v, np.float32), (128, D))
    )
    common = {
        "xT": xT,
        "w1": np.ascontiguousarray(np.asarray(W1, np.float32).astype(bf16)),
        "w2": np.ascontiguousarray(np.asarray(W2, np.float32).astype(bf16)),
        "b1c": np.ascontiguousarray(np.asarray(b1, np.float32).reshape(MH, 128).T),
        "b2r": np.ascontiguousarray(np.asarray(b2, np.float32).reshape(1, D)),
        "g1bc": bcast(ln1_g), "h1bc": bcast(ln1_b),
        "g2bc": bcast(ln2_g), "h2bc": bcast(ln2_b),
        "ident": np.eye(128, dtype=np.float32),
        "identb": np.eye(128, dtype=np.float32).astype(bf16),
        "ones128": np.ones((1, 128), np.float32),
        "onesv": np.ones((128, 64), np.float32),
    }
    in_maps = []
    for c in range(8):
        h0 = 2 * c
        m = dict(common)
        m["wq"] = np.ascontiguousarray(
            Wq[h0:h0 + 2].transpose(1, 0, 2).reshape(D, 128).astype(bf16)
        )
        m["wk"] = np.ascontiguousarray(
            Wk[h0:h0 + 2].transpose(1, 0, 2).reshape(D, 128).astype(bf16)
        )
        m["wv"] = np.ascontiguousarray(
            Wv[h0:h0 + 2].transpose(1, 0, 2).reshape(D, 128).astype(bf16)
        )
        m["wo"] = np.ascontiguousarray(Wo[h0 * 64:h0 * 64 + 128, :])
        m["xs"] = np.ascontiguousarray(xf[_core_rows(c)])
        in_maps.append(m)
    return in_maps


def kernel(x, Wq, Wk, Wv, Wo, ln1_g, ln1_b, W1, b1, W2, b2, ln2_g, ln2_b):
    from concourse.bass_utils import run_bass_kernel_spmd

    if "nc" not in _CACHE:
        _CACHE["nc"] = _build()
    nc = _CACHE["nc"]
    in_maps = _in_maps(x, Wq, Wk, Wv, Wo, ln1_g, ln1_b, W1, b1, W2, b2, ln2_g, ln2_b)
    res = run_bass_kernel_spmd(nc, in_maps, core_ids=list(range(8)))
    out = np.empty((N, D), np.float32)
    for c in range(8):
        out[_core_rows(c)] = res.results[c]["y"]
    return out.reshape(B, S, D)


# revision 13
# speedup vs baseline: 1.4702x; 1.0423x over previous
# v6: head-parallel attention (2 heads/core over all 4096 tokens) — no K/V
# collective. Per core: project Q/K/V for its own heads from the full x^T
# (bf16 inputs, f32 accumulate), run attention software-pipelined so the PE
# never waits on the exp (PV for step k issues under the exp of step k+1),
# multiply by the core's Wo row slice, and combine partial attn_out with a
# ReduceScatter (4 pipelined 1024-token chunks overlapped with attention).
# Each core then owns 512 tokens (4 groups of 128): residual+LN1, FFN in
# bf16 split 384/128 tokens so FFN1 on the first three groups hides the last
# reduce-scatter, residual+LN2. DMAs are batched large and kept off the
# Activation queue during attention (its SEQ must keep dispatching exps).
import numpy as np

B, S, D = 2, 2048, 1024
H, DK, DVH, DFF = 16, 64, 64, 4096
N = B * S            # 4096 flattened tokens (b*S + s)
TOK = 512            # tokens owned per core after reduce-scatter
KC = D // 128        # 8
MH = DFF // 128      # 32
NG = 4               # reduce-scatter chunks (1024 tokens each)
EPS = 1e-5

_CACHE = {}


def _build():
    import concourse.mybir as mybir
    import concourse.tile as tile
    from concourse import bacc

    f32, f32r = mybir.dt.float32, mybir.dt.float32r
    bf16 = mybir.dt.bfloat16
    Exp = mybir.ActivationFunctionType.Exp
    Sqrt = mybir.ActivationFunctionType.Sqrt
    Ident = mybir.ActivationFunctionType.Identity
    AX = mybir.AxisListType.X
    Alu = mybir.AluOpType

    nc = bacc.Bacc("TRN2", target_bir_lowering=False, debug=False, num_devices=8)

    xT_d = nc.dram_tensor("xT", [D, N], bf16, kind="ExternalInput")
    xs_d = nc.dram_tensor("xs", [TOK, D], f32, kind="ExternalInput")
    wq_d = nc.dram_tensor("wq", [D, 128], bf16, kind="ExternalInput")
    wk_d = nc.dram_tensor("wk", [D, 128], bf16, kind="ExternalInput")
    wv_d = nc.dram_tensor("wv", [D, 128], bf16, kind="ExternalInput")
    wo_d = nc.dram_tensor("wo", [128, D], f32r, kind="ExternalInput")
    w1_d = nc.dram_tensor("w1", [D, DFF], bf16, kind="ExternalInput")
    w2_d = nc.dram_tensor("w2", [DFF, D], bf16, kind="ExternalInput")
    b1c_d = nc.dram_tensor("b1c", [128, MH], f32, kind="ExternalInput")
    b2r_d = nc.dram_tensor("b2r", [1, D], f32r, kind="ExternalInput")
    g1bc_d = nc.dram_tensor("g1bc", [128, D], f32, kind="ExternalInput")
    h1bc_d = nc.dram_tensor("h1bc", [128, D], f32, kind="ExternalInput")
    g2bc_d = nc.dram_tensor("g2bc", [128, D], f32, kind="ExternalInput")
    h2bc_d = nc.dram_tensor("h2bc", [128, D], f32, kind="ExternalInput")
    ident_d = nc.dram_tensor("ident", [128, 128], f32, kind="ExternalInput")
    identb_d = nc.dram_tensor("identb", [128, 128], bf16, kind="ExternalInput")
    ones64_d = nc.dram_tensor("ones64", [1, 64], f32r, kind="ExternalInput")
    ones128_d = nc.dram_tensor("ones128", [1, 128], f32r, kind="ExternalInput")
    onesv_d = nc.dram_tensor("onesv", [128, 64], f32r, kind="ExternalInput")
    y_d = nc.dram_tensor("y", [TOK, D], f32, kind="ExternalOutput")

    def ln_apply(pool, t, gbc, hbc, out_ap):
        sums = pool.tile([128, 1], f32, tag="ln_sums", name="ln_sums")
        nc.vector.reduce_sum(sums[:], t[:], axis=AX)
        # sq shares the ln_xa tag ring: it is written, never read.
        sq = pool.tile([128, D], f32, tag="ln_xa", name="ln_sq")
        ssq = pool.tile([128, 1], f32, tag="ln_ssq", name="ln_ssq")
        nc.scalar.activation(
            sq[:], t[:], mybir.ActivationFunctionType.Square, accum_out=ssq[:]
        )
        s2 = pool.tile([128, 1], f32, tag="ln_s2", name="ln_s2")
        nc.vector.tensor_mul(s2[:], sums[:], sums[:])
        var0 = pool.tile([128, 1], f32, tag="ln_var0", name="ln_var0")
        nc.vector.tensor_scalar(
            out=var0[:], in0=ssq[:], scalar1=1.0 / D, scalar2=EPS,
            op0=Alu.mult, op1=Alu.add,
        )
        s2b = pool.tile([128, 1], f32, tag="ln_s2b", name="ln_s2b")
        nc.vector.tensor_scalar_mul(s2b[:], s2[:], 1.0 / (D * D))
        var = pool.tile([128, 1], f32, tag="ln_var", name="ln_var")
        nc.vector.tensor_sub(var[:], var0[:], s2b[:])
        sd = pool.tile([128, 1], f32, tag="ln_sd", name="ln_sd")
        nc.scalar.activation(sd[:], var[:], Sqrt)
        rv = pool.tile([128, 1], f32, tag="ln_rv", name="ln_rv")
        nc.vector.reciprocal(rv[:], sd[:])
        nmr = pool.tile([128, 1], f32, tag="ln_nmr", name="ln_nmr")
        nc.vector.tensor_mul(nmr[:], sums[:], rv[:])
        nmr2 = pool.tile([128, 1], f32, tag="ln_nmr2", name="ln_nmr2")
        nc.vector.tensor_scalar_mul(nmr2[:], nmr[:], -1.0 / D)
        xa = pool.tile([128, D], f32, tag="ln_xa", name="ln_xa")
        nc.scalar.activation(xa[:], t[:], Ident, bias=nmr2[:], scale=rv[:])
        xg = pool.tile([128, D], f32, tag="ln_xg", name="ln_xg")
        nc.vector.tensor_mul(xg[:], xa[:], gbc[:])
        nc.vector.tensor_add(out_ap, xg[:], hbc[:])

    with tile.TileContext(nc) as tc:
        with (
            tc.tile_pool(name="const", bufs=1) as cpool,
            tc.tile_pool(name="lnp", bufs=2) as lnp,
            tc.tile_pool(name="rsp", bufs=2) as rsp,
            tc.tile_pool(name="w1pre", bufs=1) as w1pre_p,
            tc.tile_pool(name="dram", bufs=1, space="DRAM") as dram,
        ):
            x1 = cpool.tile([128, NG, D], bf16)
            x1T = cpool.tile([128, KC, TOK], bf16)

            rs_in = [
                dram.tile([1024, D], f32, name=f"rs_in{g}") for g in range(NG)
            ]
            rs_out = [
                dram.tile([128, D], f32, name=f"rs_out{g}") for g in range(NG)
            ]

            def x1_ln(g, xsrow, g1, h1):
                rst = rsp.tile([128, D], f32, tag="rst", name="rst")
                nc.sync.dma_start(rst[:], rs_out[g].opt())
                t = rsp.tile([128, D], f32, tag="t1", name="t1")
                nc.vector.tensor_add(t[:], rst[:], xsrow)
                ln_apply(lnp, t, g1, h1, x1[:, g, :])

            w1pre = [
                w1pre_p.tile([128, KC, 512], bf16, name=f"w1pre{i}")
                for i in range(2)
            ]

            with tc.tile_pool(name="qkt", bufs=1) as qkt:
                qT = qkt.tile([128, N], f32r)
                kT = qkt.tile([128, N], f32r)
                v_sb = qkt.tile([128, N // 128, 2, 65], f32r)

                # gpsimd (SWDGE) carries all constants, ordered by first use
                wv_sb = qkt.tile([128, KC, 128], bf16)
                nc.gpsimd.dma_start(
                    wv_sb[:], wv_d.ap().rearrange("(kc p) m -> p kc m", p=128)
                )
                ident = cpool.tile([128, 128], f32)
                nc.gpsimd.dma_start(ident[:], ident_d.ap())
                nc.gpsimd.dma_start(
                    v_sb[:, :, :, 64:65].squeeze(3).rearrange("p a h -> p (a h)"),
                    onesv_d.ap(),
                )
                wo_sb = cpool.tile([128, D], f32r)
                nc.gpsimd.dma_start(wo_sb[:], wo_d.ap())
                ones64 = cpool.tile([1, 64], f32r)
                nc.gpsimd.dma_start(ones64[:], ones64_d.ap())
                ones128 = cpool.tile([1, 128], f32r)
                nc.gpsimd.dma_start(ones128[:], ones128_d.ap())
                identb = cpool.tile([128, 128], bf16)
                nc.gpsimd.dma_start(identb[:], identb_d.ap())
                b1c = cpool.tile([128, MH], f32)
                nc.gpsimd.dma_start(b1c[:], b1c_d.ap())
                b2r = cpool.tile([1, D], f32r)
                nc.gpsimd.dma_start(b2r[:], b2r_d.ap())
                g1bc = cpool.tile([128, D], f32)
                nc.gpsimd.dma_start(g1bc[:], g1bc_d.ap())
                h1bc = cpool.tile([128, D], f32)
                nc.gpsimd.dma_start(h1bc[:], h1bc_d.ap())
                xs = cpool.tile([128, NG, D], f32)
                nc.gpsimd.dma_start(
                    xs[:], xs_d.ap().rearrange("(g p) d -> p g d", p=128)
                )

                # ---- Phase 1: Q/K/V projections over all tokens
                with (
                    tc.tile_pool(name="wqk", bufs=1) as wqk,
                    tc.tile_pool(name="vtm", bufs=2) as vtm,
                    tc.tile_pool(name="xk", bufs=2) as xkp,
                    tc.tile_pool(name="ps_p", bufs=2, space="PSUM") as ps_p,
                    tc.tile_pool(name="ps_tr", bufs=2, space="PSUM") as ps_tr,
                ):
                    wq_sb = wqk.tile([128, KC, 128], bf16)
                    nc.sync.dma_start(
                        wq_sb[:], wq_d.ap().rearrange("(kc p) m -> p kc m", p=128)
                    )
                    wk_sb = wqk.tile([128, KC, 128], bf16)
                    nc.scalar.dma_start(
                        wk_sb[:], wk_d.ap().rearrange("(kc p) m -> p kc m", p=128)
                    )
                    for tg in range(8):
                        xk = xkp.tile([128, KC, 512], bf16, tag="xk",
                                      name=f"xk{tg}")
                        (nc.sync if tg % 2 == 0 else nc.scalar).dma_start(
                            xk[:],
                            xT_d.ap()[:, tg * 512:(tg + 1) * 512]
                            .rearrange("(kc p) t -> p kc t", p=128),
                        )
                        ps = ps_p.tile([128, 3, 512], f32, tag="psp", name="psp")
                        for kc in range(KC):
                            nc.tensor.matmul(
                                ps[:, 0, :], wq_sb[:, kc, :], xk[:, kc, :],
                                start=(kc == 0), stop=(kc == KC - 1),
                            )
                            nc.tensor.matmul(
                                ps[:, 1, :], wk_sb[:, kc, :], xk[:, kc, :],
                                start=(kc == 0), stop=(kc == KC - 1),
                            )
                            nc.tensor.matmul(
                                ps[:, 2, :], wv_sb[:, kc, :], xk[:, kc, :],
                                start=(kc == 0), stop=(kc == KC - 1),
                            )
                        nc.vector.tensor_copy(
                            qT[:, tg * 512:(tg + 1) * 512], ps[:, 0, :]
                        )
                        nc.vector.tensor_copy(
                            kT[:, tg * 512:(tg + 1) * 512], ps[:, 1, :]
                        )
                        vtmp = vtm.tile([128, 512], f32, tag="vtmp", name="vtmp")
                        nc.vector.tensor_copy(vtmp[:], ps[:, 2, :])
                        for tb in range(4):
                            pt = ps_tr.tile([128, 128], f32, tag="trp", name="trp")
                            nc.tensor.transpose(
                                pt[:], vtmp[:, tb * 128:(tb + 1) * 128], ident[:]
                            )
                            nc.vector.tensor_copy(
                                v_sb[:, tg * 4 + tb, :, 0:64],
                                pt[:].rearrange("p (h v) -> p h v", h=2),
                            )

                # ---- Phase 2: attention + Wo partials + pipelined RS
                with (
                    tc.tile_pool(name="at", bufs=3) as atpool,
                    tc.tile_pool(name="onq", bufs=2) as onqp,
                    tc.tile_pool(name="wosb", bufs=2) as wosp,
                    tc.tile_pool(name="rcp", bufs=1) as rcp,
                    tc.tile_pool(name="ps_s", bufs=2, space="PSUM") as ps_s,
                    tc.tile_pool(name="ps_o", bufs=2, space="PSUM") as ps_o,
                    tc.tile_pool(name="ps_r", bufs=1, space="PSUM") as ps_r,
                    tc.tile_pool(name="ps_w", bufs=1, space="PSUM") as ps_w,
                ):
                    for qc in range(8):
                        b = qc // 4
                        po = [
                            ps_o.tile([65, 512], f32, tag="po", name=f"po{qc}_{hh}")
                            for hh in range(2)
                        ]

                        def pv(prev):
                            pat, pg, phh = prev
                            for j in range(2):
                                kt = 2 * pg + j
                                nc.tensor.matmul(
                                    po[phh][:],
                                    v_sb[:, b * 16 + kt, phh, :],
                                    pat[:, j, :],
                                    start=(kt == 0), stop=(kt == 15),
                                )

                        prev = None
                        for g in range(8):
                            for hh in range(2):
                                sT = ps_s.tile([128, 2, 512], f32, tag="sT",
                                               name="sT")
                                for j in range(2):
                                    kt = 2 * g + j
                                    nc.tensor.matmul(
                                        sT[:, j, :],
                                        kT[hh * 64:(hh + 1) * 64,
                                           b * S + kt * 128:b * S + (kt + 1) * 128],
                                        qT[hh * 64:(hh + 1) * 64,
                                           qc * 512:(qc + 1) * 512],
                                        tile_position=(hh * 64, 0),
                                    )
                                at = atpool.tile([128, 2, 512], f32r, tag="at",
                                                 name="at")
                                nc.scalar.activation(
                                    at[:], sT[:], Exp, scale=0.125
                                )
                                if prev is not None:
                                    pv(prev)
                                prev = (at, g, hh)
                        pv(prev)

                        o_nq = onqp.tile([128, 512], f32r, tag="onq", name="onq")
                        for hh in range(2):
                            rec = rcp.tile([1, 512], f32r, tag="rec", name="rec")
                            with nc.allow_low_precision(reason="f32r"):
                                nc.vector.reciprocal(rec[:], po[hh][64:65, :])
                            rp = ps_r.tile([64, 512], f32, tag="rp", name="rp")
                            nc.tensor.matmul(rp[:], ones64[:], rec[:])
                            rsb = rcp.tile([64, 512], f32, tag="rsb", name="rsb")
                            nc.vector.tensor_copy(rsb[:], rp[:])
                            nc.vector.tensor_mul(
                                o_nq[hh * 64:(hh + 1) * 64, :],
                                po[hh][0:64, :],
                                rsb[:],
                            )
                        for half in range(2):
                            wos = wosp.tile([128, 2, D], f32, tag="wos",
                                            name="wos")
                            for sub in range(2):
                                tc4 = half * 2 + sub
                                for ncc in range(2):
                                    psw = ps_w.tile([128, 512], f32, tag="psw",
                                                    name="psw")
                                    nc.tensor.matmul(
                                        psw[:],
                                        o_nq[:, tc4 * 128:(tc4 + 1) * 128],
                                        wo_sb[:, ncc * 512:(ncc + 1) * 512],
                                    )
                                    nc.vector.tensor_copy(
                                        wos[:, sub, ncc * 512:(ncc + 1) * 512],
                                        psw[:],
                                    )
                            nc.sync.dma_start(
                                rs_in[qc // 2]
                                [(qc % 2) * 512 + half * 256:
                                 (qc % 2) * 512 + (half + 1) * 256, :]
                                .rearrange("(a p) d -> p a d", p=128),
                                wos[:],
                            )
                        if qc % 2 == 1:
                            nc.gpsimd.collective_compute(
                                "ReduceScatter",
                                Alu.add,
                                ins=[rs_in[qc // 2].opt()],
                                outs=[rs_out[qc // 2].opt()],
                                replica_groups=[[0, 1, 2, 3, 4, 5, 6, 7]],
                            )
                        if qc in (5, 6):
                            i = qc - 5
                            nc.gpsimd.dma_start(
                                w1pre[i][:],
                                w1_d.ap()[:, i * 512:(i + 1) * 512]
                                .rearrange("(kc p) m -> p kc m", p=128),
                            )

            # ---- Phase 3 + 4: LN1 (groups 0-2), x1 transpose, FFN

            with (
                tc.tile_pool(name="w1p", bufs=4) as w1p,
                tc.tile_pool(name="htp", bufs=1) as htp,
                tc.tile_pool(name="w2p", bufs=3) as w2p,
                tc.tile_pool(name="ffc", bufs=1) as ffc,
            ):
                g2bc = ffc.tile([128, D], f32)
                nc.gpsimd.dma_start(g2bc[:], g2bc_d.ap())
                h2bc = ffc.tile([128, D], f32)
                nc.gpsimd.dma_start(h2bc[:], h2bc_d.ap())
                hT = htp.tile([128, MH, TOK], bf16)
                w2pre = []
                w1tiles = {}

                def f1b(bg, ps_f1b):
                    bt = w1tiles.pop(bg)
                    for a in range(4):
                        mh = 4 * bg + a
                        psb = ps_f1b.tile([128, 128], f32, tag="psf1b",
                                          name="psf1b")
                        for dc in range(KC):
                            nc.tensor.matmul(
                                psb[:], bt[:, dc, a * 128:(a + 1) * 128],
                                x1T[:, dc, 384:512],
                                start=(dc == 0), stop=(dc == KC - 1),
                            )
                        nc.vector.tensor_scalar(
                            out=hT[:, mh, 384:512], in0=psb[:],
                            scalar1=b1c[:, mh:mh + 1], scalar2=0.0,
                            op0=Alu.add, op1=Alu.max,
                        )

                with (
                    tc.tile_pool(name="ps_t2", bufs=2, space="PSUM") as ps_t2,
                    tc.tile_pool(name="ps_f1a", bufs=2, space="PSUM") as ps_f1a,
                    tc.tile_pool(name="ps_f1b", bufs=2, space="PSUM") as ps_f1b,
                ):
                    def x1_transpose(g):
                        for dc in range(KC):
                            pt2 = ps_t2.tile([128, 128], bf16, tag="trp2",
                                             name="trp2")
                            nc.tensor.transpose(
                                pt2[:], x1[:, g, dc * 128:(dc + 1) * 128],
                                identb[:],
                            )
                            nc.vector.tensor_copy(
                                x1T[:, dc, g * 128:(g + 1) * 128], pt2[:]
                            )

                    for g in range(3):
                        x1_ln(g, xs[:, g, :], g1bc, h1bc)
                        x1_transpose(g)

                    for g4 in range(8):
                        if g4 < 2:
                            w1t = w1pre[g4]
                        else:
                            w1t = w1p.tile([128, KC, 512], bf16, tag="w1t",
                                           name="w1t")
                            (nc.sync if g4 % 2 == 0 else nc.scalar).dma_start(
                                w1t[:],
                                w1_d.ap()[:, g4 * 512:(g4 + 1) * 512]
                                .rearrange("(kc p) m -> p kc m", p=128),
                            )
                        w1tiles[g4] = w1t
                        for a in range(4):
                            mh = 4 * g4 + a
                            psa = ps_f1a.tile([128, 512], f32, tag="psf1a",
                                              name="psf1a")
                            for dc in range(KC):
                                nc.tensor.matmul(
                                    psa[:, 0:384],
                                    w1t[:, dc, a * 128:(a + 1) * 128],
                                    x1T[:, dc, 0:384],
                                    start=(dc == 0), stop=(dc == KC - 1),
                                )
                            nc.vector.tensor_scalar(
                                out=hT[:, mh, 0:384], in0=psa[:, 0:384],
                                scalar1=b1c[:, mh:mh + 1], scalar2=0.0,
                                op0=Alu.add, op1=Alu.max,
                            )
                        if g4 == 3:
                            x1_ln(3, xs[:, 3, :], g1bc, h1bc)
                            x1_transpose(3)
                        if g4 >= 3:
                            f1b(g4 - 3, ps_f1b)
                        if g4 == 6:
                            w2t = w2p.tile([128, 4, D], bf16, tag="w2t",
                                           name="w2t")
                            nc.gpsimd.dma_start(
                                w2t[:],
                                w2_d.ap()[0:512, :]
                                .rearrange("(a p) d -> p a d", p=128),
                            )
                            w2pre.append(w2t)
                    for bg in range(5, 8):
                        f1b(bg, ps_f1b)

                # ---- FFN2 + residual + LN2
                with (
                    tc.tile_pool(name="ps_f2", bufs=1, space="PSUM") as ps_f2,
                    tc.tile_pool(name="outp", bufs=1) as outp,
                ):
                    psy = [
                        [
                            ps_f2.tile([128, 512], f32, tag=f"py{mt}{ncc}",
                                       name=f"py{mt}{ncc}")
                            for ncc in range(2)
                        ]
                        for mt in range(4)
                    ]
                    def f2_finish(mt):
                        for ncc in range(2):
                            nc.tensor.matmul(
                                psy[mt][ncc][:],
                                ones128[:],
                                b2r[:, ncc * 512:(ncc + 1) * 512],
                                start=False, stop=True,
                            )
                        t2 = outp.tile([128, D], f32, tag="t2", name="t2")
                        for ncc in range(2):
                            nc.vector.tensor_add(
                                t2[:, ncc * 512:(ncc + 1) * 512],
                                psy[mt][ncc][:],
                                x1[:, mt, ncc * 512:(ncc + 1) * 512],
                            )
                        ot = outp.tile([128, D], f32, tag="ot", name="ot")
                        ln_apply(lnp, t2, g2bc, h2bc, ot[:])
                        nc.sync.dma_start(
                            y_d.ap()[mt * 128:(mt + 1) * 128, :], ot[:]
                        )

                    for p2 in range(2):
                        for wc in range(8):
                            if p2 == 0 and wc < len(w2pre):
                                w2t = w2pre[wc]
                            else:
                                w2t = w2p.tile([128, 4, D], bf16, tag="w2t",
                                               name="w2t")
                                (nc.sync if wc % 2 == 0 else nc.scalar).dma_start(
                                    w2t[:],
                                    w2_d.ap()[wc * 512:(wc + 1) * 512, :]
                                    .rearrange("(a p) d -> p a d", p=128),
                                )
                            for a in range(4):
                                mh = 4 * wc + a
                                for mt in (2 * p2, 2 * p2 + 1):
                                    for ncc in range(2):
                                        nc.tensor.matmul(
                                            psy[mt][ncc][:],
                                            hT[:, mh, mt * 128:(mt + 1) * 128],
                                            w2t[:, a, ncc * 512:(ncc + 1) * 512],
                                            start=(mh == 0), stop=False,
                                        )
                        f2_finish(2 * p2)
                        f2_finish(2 * p2 + 1)
    nc.compile()
    return nc


def _core_rows(c):
    return (np.arange(NG)[:, None] * 1024 + c * 128 + np.arange(128)[None, :]).ravel()


def _in_maps(x, Wq, Wk, Wv, Wo, ln1_g, ln1_b, W1, b1, W2, b2, ln2_g, ln2_b):
    import ml_dtypes

    bf16 = ml_dtypes.bfloat16
    xf = np.ascontiguousarray(np.asarray(x, np.float32).reshape(N, D))
    xT = np.ascontiguousarray(xf.T.astype(bf16))
    Wq = np.asarray(Wq, np.float32)
    Wk = np.asarray(Wk, np.float32)
    Wv = np.asarray(Wv, np.float32)
    Wo = np.asarray(Wo, np.float32)
    bcast = lambda v: np.ascontiguousarray(
        np.broadcast_to(np.asarray(v, np.float32), (128, D))
    )
    common = {
        "xT": xT,
        "w1": np.ascontiguousarray(np.asarray(W1, np.float32).astype(bf16)),
        "w2": np.ascontiguousarray(np.asarray(W2, np.float32).astype(bf16)),
        "b1c": np.ascontiguousarray(np.asarray(b1, np.float32).reshape(MH, 128).T),
        "b2r": np.ascontiguousarray(np.asarray(b2, np.float32).reshape(1, D)),
        "g1bc": bcast(ln1_g), "h1bc": bcast(ln1_b),
        "g2bc": bcast(ln2_g), "h2bc": bcast(ln2_b),
        "ident": np.eye(128, dtype=np.float32),
        "identb": np.eye(128, dtype=np.float32).astype(bf16),
        "ones64": np.ones((1, 64), np.float32),
        "ones128": np.ones((1, 128), np.float32),
        "onesv": np.ones((128, 64), np.float32),
    }
    in_maps = []
    for c in range(8):
        h0 = 2 * c
        m = dict(common)
        m["wq"] = np.ascontiguousarray(
            Wq[h0:h0 + 2].transpose(1, 0, 2).reshape(D, 128).astype(bf16)
        )
        m["wk"] = np.ascontiguousarray(
            Wk[h0:h0 + 2].transpose(1, 0, 2).reshape(D, 128).astype(bf16)
        )
        m["wv"] = np.ascontiguousarray(
            Wv[h0:h0 + 2].transpose(1, 0, 2).reshape(D, 128).astype(bf16)
        )
        m["wo"] = np.ascontiguousarray(Wo[h0 * 64:h0 * 64 + 128, :])
        m["xs"] = np.ascontiguousarray(xf[_core_rows(c)])
        in_maps.append(m)
    return in_maps


def kernel(x, Wq, Wk, Wv, Wo, ln1_g, ln1_b, W1, b1, W2, b2, ln2_g, ln2_b):
    from concourse.bass_utils import run_bass_kernel_spmd

    if "nc" not in _CACHE:
        _CACHE["nc"] = _build()
    nc = _CACHE["nc"]
    in_maps = _in_maps(x, Wq, Wk, Wv, Wo, ln1_g, ln1_b, W1, b1, W2, b2, ln2_g, ln2_b)
    res = run_bass_kernel_spmd(nc, in_maps, core_ids=list(range(8)))
    out = np.empty((N, D), np.float32)
    for c in range(8):
        out[_core_rows(c)] = res.results[c]["y"]
    return out.reshape(B, S, D)
